# revision 27
# baseline (speedup 1.0000x reference)
"""MetaOptNet SVM-CS classification head on 8 Trainium2 NeuronCores.

Sharding: data-parallel over query rows (2048 -> 8 x 256). Each core:
  - computes the support Gram matrix K = S S^T (contraction over d=8192
    via PE-transposed support chunks), replicated;
  - runs the full interior-point QP (14 Newton steps) replicated,
    exploiting that the KKT matrix H = kron(K,I5)+I+diag(lam/s) is
    block-diagonal over the 5 classes: five 125x125 SPD solves plus a
    125x125 Schur complement for the equality constraints. All solves
    use Newton-Schulz inversion with fresh Jacobi initialization
    (rho(I - X0 H) <= ~0.3 uniformly along the trajectory);
  - streams its query slice, PE-transposes chunks, accumulates
    compat = S Q_c^T in PSUM (overlaps the QP);
  - logits_c = compat^T @ qp3, scaled by `scale`.
Host only shards inputs, concatenates the 8 logits slices, and casts
num_sv (computed on device) to int32.
"""
import sys

for _p in ("/opt/trn_rl_repo", "/root/.axon_site/_ro/trn_rl_repo"):
    if _p not in sys.path:
        sys.path.append(_p)

import numpy as np

import concourse.bacc as bacc
import concourse.mybir as mybir
import concourse.tile as tile
from concourse.bass_utils import run_bass_kernel_spmd
from concourse.masks import make_identity

F32 = mybir.dt.float32
ALU = mybir.AluOpType
AXL = mybir.AxisListType

N_CORES = 8
NQ, D, NS, NW = 2048, 8192, 125, 5
NQC = NQ // N_CORES          # 256 query rows per core
DC = D // 128                # 64 d-chunks
C_REG, SIGMA = 0.1, 0.1
# Per-iteration (reinit, newton_schulz_steps) for the H/Schur inverses:
# fresh Jacobi + 2 NS through the lam/s spike, warm-started 1 NS while it
# decays, frozen inverses for the converged tail.
NS_SCHED = [(True, 2)] * 4 + [(False, 1)] * 6 + [(False, 0)] * 3
IP_ITERS = len(NS_SCHED)
SV_THRESH = 0.001


def build_nc():
    nc = bacc.Bacc("TRN2", target_bir_lowering=False, debug=False,
                   num_devices=N_CORES)
    q_dram = nc.dram_tensor("qslice", [NQC, D], F32, kind="ExternalInput")
    s_dram = nc.dram_tensor("support", [NS, D], F32, kind="ExternalInput")
    y_dram = nc.dram_tensor("y1h", [NS, NW], F32, kind="ExternalInput")
    sc_dram = nc.dram_tensor("scale", [1, 1], F32, kind="ExternalInput")
    lg_dram = nc.dram_tensor("logits", [NQC, NW], F32, kind="ExternalOutput")
    aux_dram = nc.dram_tensor("aux", [1, 4], F32, kind="ExternalOutput")

    with tile.TileContext(nc) as tc:
        _build(tc, q_dram, s_dram, y_dram, sc_dram, lg_dram, aux_dram)
    nc.compile()
    return nc


def _build(tc, q_dram, s_dram, y_dram, sc_dram, lg_dram, aux_dram):
    nc = tc.nc
    from contextlib import ExitStack

    ctx = ExitStack()
    with ctx:
        persist = ctx.enter_context(tc.tile_pool(name="persist", bufs=1))
        qp_tmp = ctx.enter_context(tc.tile_pool(name="qp_tmp", bufs=2))
        ns_sb = ctx.enter_context(tc.tile_pool(name="ns_sb", bufs=2))
        qio = ctx.enter_context(tc.tile_pool(name="qio", bufs=3))
        qtp = ctx.enter_context(tc.tile_pool(name="qtp", bufs=3))
        tr_ps = ctx.enter_context(tc.tile_pool(name="tr_ps", bufs=2, space="PSUM"))
        mm_ps = ctx.enter_context(tc.tile_pool(name="mm_ps", bufs=1, space="PSUM"))
        y_ps = ctx.enter_context(tc.tile_pool(name="y_ps", bufs=2, space="PSUM"))
        xy_ps = ctx.enter_context(tc.tile_pool(name="xy_ps", bufs=2, space="PSUM"))
        cp_ps = ctx.enter_context(tc.tile_pool(name="cp_ps", bufs=1, space="PSUM"))

        # ---------------- constants ----------------
        ident = persist.tile([128, 128], F32)
        make_identity(nc, ident)
        ones_sq = persist.tile([NS, NS], F32)
        nc.gpsimd.memset(ones_sq[:], 1.0)
        ones_row = persist.tile([1, 128], F32)
        nc.gpsimd.memset(ones_row[:], 1.0)

        y1 = persist.tile([NS, NW], F32)
        nc.sync.dma_start(y1[:], y_dram[:])
        hC = persist.tile([NS, NW], F32)
        nc.vector.tensor_scalar_mul(hC[:], y1[:], C_REG)

        sc_sb = persist.tile([1, 1], F32)
        nc.sync.dma_start(sc_sb[:], sc_dram[:])
        scp = mm_ps.tile([128, 1], F32, tag="mm_small")
        nc.tensor.matmul(scp[:], ones_row[:], sc_sb[:], start=True, stop=True)
        sc_bc = persist.tile([128, 1], F32)
        nc.scalar.copy(sc_bc[:], scp[:])

        # compat + K share one PSUM bank: cols 0:256 compat, 256:381 K
        cpt = cp_ps.tile([NS, NQC + NS], F32)

        # ---------------- phase A: support load, S^T, K ----------------
        stq = persist.tile([128, DC * NS], F32)      # S^T chunks [128d, 125]
        with tc.tile_pool(name="snat", bufs=1) as snat_pool:
            s_nat = snat_pool.tile([NS, D], F32)
            nc.sync.dma_start(s_nat[:], s_dram[:])
            for c in range(DC):
                tp = tr_ps.tile([128, NS], F32)
                nc.tensor.transpose(tp[:], s_nat[:, c * 128:(c + 1) * 128],
                                    ident[:NS, :NS])
                if c % 2 == 0:
                    nc.scalar.copy(stq[:, c * NS:(c + 1) * NS], tp[:])
                else:
                    nc.vector.tensor_copy(stq[:, c * NS:(c + 1) * NS], tp[:])
            kp = cpt[:, NQC:NQC + NS]
            for c in range(DC):
                nc.tensor.matmul(kp, stq[:, c * NS:(c + 1) * NS],
                                 stq[:, c * NS:(c + 1) * NS],
                                 start=(c == 0), stop=(c == DC - 1))
            K_sb = persist.tile([NS, NS], F32)
            nc.scalar.copy(K_sb[:], kp)

        # Kzd = K with zeroed diagonal; diagKp1 = diag(K) + 1
        kdiag = persist.tile([NS, NS], F32)
        dK = persist.tile([NS, 1], F32)
        nc.vector.scalar_tensor_tensor(kdiag[:], K_sb[:], 1.0, ident[:NS, :NS],
                                       op0=ALU.mult, op1=ALU.mult,
                                       accum_out=dK[:])
        Kzd = persist.tile([NS, NS], F32)
        nc.vector.tensor_tensor(Kzd[:], K_sb[:], kdiag[:], op=ALU.subtract)
        dKp1 = persist.tile([NS, 1], F32)
        nc.vector.tensor_scalar_add(dKp1[:], dK[:], 1.0)

        # ---------------- phase C emit helper (independent of QP) -------
        # Both query row-blocks transposed into one [128d, 256q] rhs per
        # d-chunk -> a single compat matmul per chunk (Sᵀ weights loaded once)
        def emit_stream():
            for bc in range(4):
                qins = []
                for qt in range(2):
                    qin = qio.tile([128, 2048], F32, tag=f"qin{qt}")
                    nc.sync.dma_start(
                        qin[:], q_dram[qt * 128:(qt + 1) * 128,
                                       bc * 2048:(bc + 1) * 2048])
                    qins.append(qin)
                for sub in range(16):
                    c = bc * 16 + sub
                    qt_sb = qtp.tile([128, 256], F32)
                    for qt in range(2):
                        tp = tr_ps.tile([128, 128], F32)
                        nc.tensor.transpose(
                            tp[:], qins[qt][:, sub * 128:(sub + 1) * 128],
                            ident[:])
                        if (2 * c + qt) % 2 == 0:
                            nc.scalar.copy(qt_sb[:, qt * 128:(qt + 1) * 128],
                                           tp[:])
                        else:
                            nc.vector.tensor_copy(
                                qt_sb[:, qt * 128:(qt + 1) * 128], tp[:])
                    nc.tensor.matmul(cpt[:, 0:NQC],
                                     stq[:, c * NS:(c + 1) * NS], qt_sb[:],
                                     start=(c == 0), stop=(c == DC - 1))

        emit_stream()

        # ---------------- phase B: interior-point QP ----------------
        # state ST: cols 0:5 Z | 5:10 S | 10:15 L | 15:16 nu
        ST = persist.tile([NS, 16], F32)
        nc.gpsimd.memset(ST[:, 0:5], 0.0)
        nc.gpsimd.memset(ST[:, 5:15], 1.0)
        nc.gpsimd.memset(ST[:, 15:16], 0.0)
        DP = persist.tile([NS, 16], F32)
        X = persist.tile([NS, NW * NS], F32)         # five H_w^{-1}
        Hs = persist.tile([NS, NW * NS], F32)
        XS = persist.tile([NS, NS], F32)
        Ssch = persist.tile([NS, NS], F32)

        for it in range(IP_ITERS):
            Zc, Sc, Lc, nuc = ST[:, 0:5], ST[:, 5:10], ST[:, 10:15], ST[:, 15:16]
            # GZ = K @ Z   (K symmetric)
            gzp = mm_ps.tile([NS, NW], F32, tag="mm_small")
            nc.tensor.matmul(gzp[:], K_sb[:], Zc, start=True, stop=True)
            # SL = S*L with row sums
            sl = qp_tmp.tile([NS, NW], F32, tag="sl")
            slsum = qp_tmp.tile([NS, 1], F32, tag="slsum")
            nc.vector.scalar_tensor_tensor(sl[:], Sc, 1.0, Lc,
                                           op0=ALU.mult, op1=ALU.mult,
                                           accum_out=slsum[:])
            # mu = SIGMA/(ns*nw) * total(SL), broadcast to all partitions
            mup = mm_ps.tile([NS, 1], F32, tag="mm_small")
            nc.tensor.matmul(mup[:], ones_sq[:], slsum[:], start=True, stop=True)
            mu = qp_tmp.tile([NS, 1], F32, tag="mu")
            nc.scalar.mul(mu[:], mup[:], SIGMA / (NS * NW))
            # r3 = SL - mu
            r3 = qp_tmp.tile([NS, NW], F32, tag="r3")
            nc.vector.tensor_scalar(r3[:], sl[:], mu[:], None, op0=ALU.subtract)
            # sinv, dv = L/s, dkd = diag(K)+1+dv, g2 = 1/dkd
            sinv = qp_tmp.tile([NS, NW], F32, tag="sinv")
            nc.vector.reciprocal(sinv[:], Sc)
            fresh, n_ns = NS_SCHED[it]
            if n_ns > 0:
                dv = qp_tmp.tile([NS, NW], F32, tag="dv")
                nc.vector.tensor_tensor(dv[:], Lc, sinv[:], op=ALU.mult)
                dkd = qp_tmp.tile([NS, NW], F32, tag="dkd")
                nc.vector.tensor_scalar(dkd[:], dv[:], dKp1[:], None,
                                        op0=ALU.add)
            if fresh:
                g2 = qp_tmp.tile([NS, NW], F32, tag="g2")
                nc.vector.reciprocal(g2[:], dkd[:])
            # r1 = GZ + Z - y1h + L + nu
            r1 = qp_tmp.tile([NS, NW], F32, tag="r1")
            nc.vector.scalar_tensor_tensor(r1[:], gzp[:], nuc, Zc,
                                           op0=ALU.add, op1=ALU.add)
            nc.vector.tensor_tensor(r1[:], r1[:], y1[:], op=ALU.subtract)
            nc.vector.tensor_tensor(r1[:], r1[:], Lc, op=ALU.add)
            # r2 = Z + S - h
            r2 = qp_tmp.tile([NS, NW], F32, tag="r2")
            nc.vector.scalar_tensor_tensor(r2[:], Zc, 1.0, Sc,
                                           op0=ALU.mult, op1=ALU.add)
            nc.vector.tensor_tensor(r2[:], r2[:], hC[:], op=ALU.subtract)
            # r4 = rowsum(Z)
            r4 = qp_tmp.tile([NS, 1], F32, tag="r4")
            nc.vector.tensor_reduce(r4[:], Zc, axis=AXL.X, op=ALU.add)
            # rhs1 = -(r1 + (L*r2 - r3)/s)
            t5 = qp_tmp.tile([NS, NW], F32, tag="t5")
            nc.vector.tensor_tensor(t5[:], Lc, r2[:], op=ALU.mult)
            nc.vector.tensor_tensor(t5[:], t5[:], r3[:], op=ALU.subtract)
            nc.vector.tensor_tensor(t5[:], t5[:], sinv[:], op=ALU.mult)
            rhs1 = qp_tmp.tile([NS, NW], F32, tag="rhs1")
            nc.vector.scalar_tensor_tensor(rhs1[:], t5[:], -1.0, r1[:],
                                           op0=ALU.mult, op1=ALU.subtract)
            # H_w = Kzd + diag(dkd_w); X0_w = diag(g2_w) on fresh iters
            for w in range(NW):
                ws = slice(w * NS, (w + 1) * NS)
                if n_ns > 0:
                    nc.vector.scalar_tensor_tensor(
                        Hs[:, ws], ident[:NS, :NS], dkd[:, w:w + 1], Kzd[:],
                        op0=ALU.mult, op1=ALU.add)
                if fresh:
                    nc.vector.tensor_scalar(X[:, ws], ident[:NS, :NS],
                                            g2[:, w:w + 1], None, op0=ALU.mult)
            # Newton-Schulz: X <- 2X - X (H X)
            for _ in range(n_ns):
                for w in range(NW):
                    ws = slice(w * NS, (w + 1) * NS)
                    yp = y_ps.tile([NS, NS], F32)
                    nc.tensor.matmul(yp[:], Hs[:, ws], X[:, ws],
                                     start=True, stop=True)
                    ysb = ns_sb.tile([NS, NS], F32, tag="ysb")
                    if w % 2 == 0:
                        nc.vector.tensor_copy(ysb[:], yp[:])
                    else:
                        nc.scalar.copy(ysb[:], yp[:])
                    xyp = xy_ps.tile([NS, NS], F32)
                    nc.tensor.matmul(xyp[:], X[:, ws], ysb[:],
                                     start=True, stop=True)
                    nc.vector.scalar_tensor_tensor(X[:, ws], X[:, ws], 2.0,
                                                   xyp[:], op0=ALU.mult,
                                                   op1=ALU.subtract)
            # usum = sum_w X_w rhs1_w (PSUM-accumulated across the 5 mms)
            up = mm_ps.tile([NS, 1], F32, tag="mm_small")
            for w in range(NW):
                nc.tensor.matmul(up[:], X[:, w * NS:(w + 1) * NS],
                                 rhs1[:, w:w + 1], start=(w == 0),
                                 stop=(w == NW - 1))
            # Schur = sum_w X_w ; Jacobi init for its inverse
            if n_ns > 0:
                xv = X[:].rearrange("p (w j) -> p j w", w=NW)
                nc.vector.tensor_reduce(Ssch[:], xv, axis=AXL.X, op=ALU.add)
            if fresh:
                dSg = qp_tmp.tile([NS, 1], F32, tag="dSg")
                schd = qp_tmp.tile([NS, NS], F32, tag="schd")
                nc.vector.scalar_tensor_tensor(schd[:], Ssch[:], 1.0,
                                               ident[:NS, :NS], op0=ALU.mult,
                                               op1=ALU.mult, accum_out=dSg[:])
                gs2 = qp_tmp.tile([NS, 1], F32, tag="gs2")
                nc.vector.reciprocal(gs2[:], dSg[:])
                nc.vector.tensor_scalar(XS[:], ident[:NS, :NS], gs2[:], None,
                                        op0=ALU.mult)
            for _ in range(n_ns):
                yp = y_ps.tile([NS, NS], F32)
                nc.tensor.matmul(yp[:], Ssch[:], XS[:], start=True, stop=True)
                ysb = ns_sb.tile([NS, NS], F32, tag="ysb")
                nc.scalar.copy(ysb[:], yp[:])
                xyp = xy_ps.tile([NS, NS], F32)
                nc.tensor.matmul(xyp[:], XS[:], ysb[:], start=True, stop=True)
                nc.vector.scalar_tensor_tensor(XS[:], XS[:], 2.0, xyp[:],
                                               op0=ALU.mult, op1=ALU.subtract)
            # rhs_s = usum + r4 ; dnu = XS rhs_s
            rhss = qp_tmp.tile([NS, 1], F32, tag="rhss")
            nc.vector.tensor_tensor(rhss[:], up[:], r4[:], op=ALU.add)
            dnup = mm_ps.tile([NS, 1], F32, tag="mm_small")
            nc.tensor.matmul(dnup[:], XS[:], rhss[:], start=True, stop=True)
            nc.scalar.copy(DP[:, 15:16], dnup[:])
            # dZ = X (rhs1 - dnu broadcast over w)
            rhs2 = qp_tmp.tile([NS, NW], F32, tag="rhs2")
            nc.vector.tensor_scalar(rhs2[:], rhs1[:], DP[:, 15:16], None,
                                    op0=ALU.subtract)
            xdp = mm_ps.tile([NS, NW], F32, tag="mm_small")
            for w in range(NW):
                nc.tensor.matmul(xdp[:, w:w + 1], X[:, w * NS:(w + 1) * NS],
                                 rhs2[:, w:w + 1], start=True, stop=True)
            nc.scalar.copy(DP[:, 0:5], xdp[:])
            # dS = -dZ - r2 ; dL = -(L*dS + r3)/s
            nc.vector.scalar_tensor_tensor(DP[:, 5:10], DP[:, 0:5], -1.0,
                                           r2[:], op0=ALU.mult,
                                           op1=ALU.subtract)
            dl = qp_tmp.tile([NS, NW], F32, tag="dl")
            nc.vector.tensor_tensor(dl[:], Lc, DP[:, 5:10], op=ALU.mult)
            nc.vector.tensor_tensor(dl[:], dl[:], r3[:], op=ALU.add)
            nc.vector.scalar_tensor_tensor(DP[:, 10:15], dl[:], -1.0, sinv[:],
                                           op0=ALU.mult, op1=ALU.mult)
            # alpha = min(1, 0.99 / max(-d(S|L)/(S|L), eps))
            xpi = qp_tmp.tile([NS, 10], F32, tag="xpi")
            nc.vector.reciprocal(xpi[:], ST[:, 5:15])
            qr_ = qp_tmp.tile([NS, 10], F32, tag="qr_")
            nc.vector.scalar_tensor_tensor(qr_[:], DP[:, 5:15], -1.0, xpi[:],
                                           op0=ALU.mult, op1=ALU.mult)
            # global max: free-reduce, one transpose, free-reduce
            qm = qp_tmp.tile([NS, 1], F32, tag="qm")
            nc.vector.tensor_reduce(qm[:], qr_[:], axis=AXL.X, op=ALU.max)
            qtr = mm_ps.tile([1, NS], F32, tag="mm_small")
            nc.tensor.transpose(qtr[:], qm[:], ident[:NS, :NS])
            qtr_sb = qp_tmp.tile([1, NS], F32, tag="qtr_sb")
            nc.scalar.copy(qtr_sb[:], qtr[:])
            am = qp_tmp.tile([1, 1], F32, tag="am")
            nc.vector.tensor_reduce(am[:], qtr_sb[:], axis=AXL.X, op=ALU.max)
            nc.vector.tensor_scalar_max(am[:], am[:], 1e-30)
            nc.vector.reciprocal(am[:], am[:])
            nc.vector.tensor_scalar(am[:], am[:], 0.99, 1.0,
                                    op0=ALU.mult, op1=ALU.min)
            alp = mm_ps.tile([NS, 1], F32, tag="mm_small")
            nc.tensor.matmul(alp[:], ones_row[:1, :NS], am[:],
                             start=True, stop=True)
            asb = qp_tmp.tile([NS, 1], F32, tag="asb")
            nc.scalar.copy(asb[:], alp[:])
            # state += alpha * delta
            nc.vector.scalar_tensor_tensor(ST[:], DP[:], asb[:], ST[:],
                                           op0=ALU.mult, op1=ALU.add)

        # ---------------- num_sv ----------------
        cmp_ = qp_tmp.tile([NS, NW], F32, tag="cmp_")
        nc.vector.tensor_scalar(cmp_[:], ST[:, 0:5], SV_THRESH, None,
                                op0=ALU.is_gt)
        anyw = qp_tmp.tile([NS, 1], F32, tag="anyw")
        nc.vector.tensor_reduce(anyw[:], cmp_[:], axis=AXL.X, op=ALU.max)
        nsp = mm_ps.tile([NS, 1], F32, tag="mm_small")
        nc.tensor.matmul(nsp[:], ones_sq[:], anyw[:], start=True, stop=True)
        aux_sb = persist.tile([1, 4], F32)
        nc.gpsimd.memset(aux_sb[:], 0.0)
        nc.scalar.copy(aux_sb[0:1, 0:1], nsp[0:1, :])
        nc.sync.dma_start(aux_dram[:], aux_sb[:])

        # ---------------- logits = compat^T @ Z, scaled ----------------
        cp_sb = persist.tile([NS, NQC], F32)
        nc.scalar.copy(cp_sb[:], cpt[:, 0:NQC])
        for qt in range(2):
            lgp = mm_ps.tile([128, NW], F32, tag="mm_small")
            nc.tensor.matmul(lgp[:], cp_sb[:, qt * 128:(qt + 1) * 128],
                             ST[:, 0:5], start=True, stop=True)
            lg_sb = qp_tmp.tile([128, NW], F32, tag="lg_sb")
            nc.vector.tensor_scalar_mul(lg_sb[:], lgp[:], sc_bc[:])
            nc.sync.dma_start(lg_dram[qt * 128:(qt + 1) * 128, :], lg_sb[:])


_NC_CACHE = None


def _get_nc():
    global _NC_CACHE
    if _NC_CACHE is None:
        _NC_CACHE = build_nc()
    return _NC_CACHE


def _prep_in_maps(query, support, support_labels, scale):
    query = np.ascontiguousarray(np.asarray(query, np.float32))
    support = np.ascontiguousarray(np.asarray(support, np.float32))
    labels = np.asarray(support_labels).astype(np.int64)
    y1h = np.zeros((NS, NW), np.float32)
    y1h[np.arange(NS), labels] = 1.0
    sc = np.asarray(scale, np.float32).reshape(1, 1)
    return [
        {
            "qslice": query[c * NQC:(c + 1) * NQC],
            "support": support,
            "y1h": y1h,
            "scale": sc,
        }
        for c in range(N_CORES)
    ]


def run_device(query, support, support_labels, scale, trace=False, **trace_kw):
    nc = _get_nc()
    in_maps = _prep_in_maps(query, support, support_labels, scale)
    res = run_bass_kernel_spmd(nc, in_maps, list(range(N_CORES)),
                               trace=trace, **trace_kw)
    logits = np.concatenate(
        [res.results[c]["logits"] for c in range(N_CORES)], axis=0)
    num_sv = res.results[0]["aux"][0, 0]
    return logits, num_sv, res


def kernel(query, support, support_labels, n_way, n_shot, scale):
    assert int(n_way) == NW and int(n_shot) * NW == NS
    logits, num_sv, _ = run_device(query, support, support_labels, scale)
    logits_full = logits.reshape(1, NQ, NW).astype(np.float32)
    return logits_full, np.int32(round(float(num_sv)))


# revision 28
# speedup vs baseline: 1.1874x; 1.1874x over previous
"""MetaOptNet SVM-CS classification head on 8 Trainium2 NeuronCores.

Sharding: data-parallel over query rows (2048 -> 8 x 256). Each core:
  - computes the support Gram matrix K = S S^T (contraction over d=8192
    via PE-transposed support chunks), replicated;
  - runs the full interior-point QP (14 Newton steps) replicated,
    exploiting that the KKT matrix H = kron(K,I5)+I+diag(lam/s) is
    block-diagonal over the 5 classes: five 125x125 SPD solves plus a
    125x125 Schur complement for the equality constraints. All solves
    use Newton-Schulz inversion with fresh Jacobi initialization
    (rho(I - X0 H) <= ~0.3 uniformly along the trajectory);
  - streams its query slice, PE-transposes chunks, accumulates
    compat = S Q_c^T in PSUM (overlaps the QP);
  - logits_c = compat^T @ qp3, scaled by `scale`.
Host only shards inputs, concatenates the 8 logits slices, and casts
num_sv (computed on device) to int32.
"""
import sys

for _p in ("/opt/trn_rl_repo", "/root/.axon_site/_ro/trn_rl_repo"):
    if _p not in sys.path:
        sys.path.append(_p)

import numpy as np

import concourse.bacc as bacc
import concourse.mybir as mybir
import concourse.tile as tile
from concourse.bass_utils import run_bass_kernel_spmd
from concourse.masks import make_identity

F32 = mybir.dt.float32
ALU = mybir.AluOpType
AXL = mybir.AxisListType

N_CORES = 8
NQ, D, NS, NW = 2048, 8192, 125, 5
NQC = NQ // N_CORES          # 256 query rows per core
DC = D // 128                # 64 d-chunks
C_REG, SIGMA = 0.1, 0.1
# Per-iteration (reinit, newton_schulz_steps) for the H/Schur inverses:
# fresh Jacobi + 2 NS through the lam/s spike, warm-started 1 NS while it
# decays, frozen inverses for the converged tail.
NS_SCHED = [(True, 2)] * 4 + [(False, 1)] * 6 + [(False, 0)] * 3
IP_ITERS = len(NS_SCHED)
SV_THRESH = 0.001


def build_nc():
    nc = bacc.Bacc("TRN2", target_bir_lowering=False, debug=False,
                   num_devices=N_CORES)
    q_dram = nc.dram_tensor("qslice", [NQC, D], F32, kind="ExternalInput")
    s_dram = nc.dram_tensor("support", [NS, D], F32, kind="ExternalInput")
    y_dram = nc.dram_tensor("y1h", [NS, NW], F32, kind="ExternalInput")
    sc_dram = nc.dram_tensor("scale", [1, 1], F32, kind="ExternalInput")
    lg_dram = nc.dram_tensor("logits", [NQC, NW], F32, kind="ExternalOutput")
    aux_dram = nc.dram_tensor("aux", [1, 4], F32, kind="ExternalOutput")

    with tile.TileContext(nc) as tc:
        _build(tc, q_dram, s_dram, y_dram, sc_dram, lg_dram, aux_dram)
    nc.compile()
    return nc


def _build(tc, q_dram, s_dram, y_dram, sc_dram, lg_dram, aux_dram):
    nc = tc.nc
    from contextlib import ExitStack

    ctx = ExitStack()
    with ctx:
        persist = ctx.enter_context(tc.tile_pool(name="persist", bufs=1))
        qp_tmp = ctx.enter_context(tc.tile_pool(name="qp_tmp", bufs=2))
        ns_sb = ctx.enter_context(tc.tile_pool(name="ns_sb", bufs=2))
        qio = ctx.enter_context(tc.tile_pool(name="qio", bufs=3))
        qtp = ctx.enter_context(tc.tile_pool(name="qtp", bufs=3))
        tr_ps = ctx.enter_context(tc.tile_pool(name="tr_ps", bufs=2, space="PSUM"))
        mm_ps = ctx.enter_context(tc.tile_pool(name="mm_ps", bufs=1, space="PSUM"))
        y_ps = ctx.enter_context(tc.tile_pool(name="y_ps", bufs=2, space="PSUM"))
        xy_ps = ctx.enter_context(tc.tile_pool(name="xy_ps", bufs=2, space="PSUM"))
        cp_ps = ctx.enter_context(tc.tile_pool(name="cp_ps", bufs=1, space="PSUM"))

        # ---------------- constants ----------------
        ident = persist.tile([128, 128], F32)
        make_identity(nc, ident)
        ones_sq = persist.tile([NS, NS], F32)
        nc.gpsimd.memset(ones_sq[:], 1.0)
        ones_row = persist.tile([1, 128], F32)
        nc.gpsimd.memset(ones_row[:], 1.0)

        y1 = persist.tile([NS, NW], F32)
        nc.sync.dma_start(y1[:], y_dram[:])
        hC = persist.tile([NS, NW], F32)
        nc.vector.tensor_scalar_mul(hC[:], y1[:], C_REG)

        sc_sb = persist.tile([1, 1], F32)
        nc.sync.dma_start(sc_sb[:], sc_dram[:])
        scp = mm_ps.tile([128, 1], F32, tag="mm_small")
        nc.tensor.matmul(scp[:], ones_row[:], sc_sb[:], start=True, stop=True)
        sc_bc = persist.tile([128, 1], F32)
        nc.scalar.copy(sc_bc[:], scp[:])

        # compat + K share one PSUM bank: cols 0:256 compat, 256:381 K
        cpt = cp_ps.tile([NS, NQC + NS], F32)

        # ---------------- phase A: support load, S^T, K ----------------
        stq = persist.tile([128, DC * NS], F32)      # S^T chunks [128d, 125]
        with tc.tile_pool(name="snat", bufs=1) as snat_pool:
            s_nat = snat_pool.tile([NS, D], F32)
            nc.sync.dma_start(s_nat[:], s_dram[:])
            for c in range(DC):
                tp = tr_ps.tile([128, NS], F32)
                nc.tensor.transpose(tp[:], s_nat[:, c * 128:(c + 1) * 128],
                                    ident[:NS, :NS])
                if c % 2 == 0:
                    nc.scalar.copy(stq[:, c * NS:(c + 1) * NS], tp[:])
                else:
                    nc.vector.tensor_copy(stq[:, c * NS:(c + 1) * NS], tp[:])
            kp = cpt[:, NQC:NQC + NS]
            for c in range(DC):
                nc.tensor.matmul(kp, stq[:, c * NS:(c + 1) * NS],
                                 stq[:, c * NS:(c + 1) * NS],
                                 start=(c == 0), stop=(c == DC - 1))
            K_sb = persist.tile([NS, NS], F32)
            nc.scalar.copy(K_sb[:], kp)

        # Kzd = K with zeroed diagonal; diagKp1 = diag(K) + 1
        kdiag = persist.tile([NS, NS], F32)
        dK = persist.tile([NS, 1], F32)
        nc.vector.scalar_tensor_tensor(kdiag[:], K_sb[:], 1.0, ident[:NS, :NS],
                                       op0=ALU.mult, op1=ALU.mult,
                                       accum_out=dK[:])
        Kzd = persist.tile([NS, NS], F32)
        nc.vector.tensor_tensor(Kzd[:], K_sb[:], kdiag[:], op=ALU.subtract)
        dKp1 = persist.tile([NS, 1], F32)
        nc.vector.tensor_scalar_add(dKp1[:], dK[:], 1.0)

        # ---------------- phase C emit helper (independent of QP) -------
        # Both query row-blocks transposed into one [128d, 256q] rhs per
        # d-chunk -> a single compat matmul per chunk (Sᵀ weights loaded once)
        def emit_stream():
            for bc in range(4):
                qins = []
                for qt in range(2):
                    qin = qio.tile([128, 2048], F32, tag=f"qin{qt}")
                    nc.sync.dma_start(
                        qin[:], q_dram[qt * 128:(qt + 1) * 128,
                                       bc * 2048:(bc + 1) * 2048])
                    qins.append(qin)
                for sub in range(16):
                    c = bc * 16 + sub
                    qt_sb = qtp.tile([128, 256], F32)
                    for qt in range(2):
                        tp = tr_ps.tile([128, 128], F32)
                        nc.tensor.transpose(
                            tp[:], qins[qt][:, sub * 128:(sub + 1) * 128],
                            ident[:])
                        if (2 * c + qt) % 2 == 0:
                            nc.scalar.copy(qt_sb[:, qt * 128:(qt + 1) * 128],
                                           tp[:])
                        else:
                            nc.vector.tensor_copy(
                                qt_sb[:, qt * 128:(qt + 1) * 128], tp[:])
                    nc.tensor.matmul(cpt[:, 0:NQC],
                                     stq[:, c * NS:(c + 1) * NS], qt_sb[:],
                                     start=(c == 0), stop=(c == DC - 1))

        emit_stream()

        # ---------------- phase B: interior-point QP ----------------
        # state ST: cols 0:5 Z | 5:10 S | 10:15 L | 15:16 nu
        ST = persist.tile([NS, 16], F32)
        nc.gpsimd.memset(ST[:, 0:5], 0.0)
        nc.gpsimd.memset(ST[:, 5:15], 1.0)
        nc.gpsimd.memset(ST[:, 15:16], 0.0)
        DP = persist.tile([NS, 16], F32)
        X = persist.tile([NS, NW * NS], F32)         # five H_w^{-1}
        Hs = persist.tile([NS, NW * NS], F32)
        XS = persist.tile([NS, NS], F32)
        Ssch = persist.tile([NS, NS], F32)

        for it in range(IP_ITERS):
            Zc, Sc, Lc, nuc = ST[:, 0:5], ST[:, 5:10], ST[:, 10:15], ST[:, 15:16]
            # GZ = K @ Z   (K symmetric)
            gzp = mm_ps.tile([NS, NW], F32, tag="mm_small")
            nc.tensor.matmul(gzp[:], K_sb[:], Zc, start=True, stop=True)
            # SL = S*L with row sums
            sl = qp_tmp.tile([NS, NW], F32, tag="sl")
            slsum = qp_tmp.tile([NS, 1], F32, tag="slsum")
            nc.vector.scalar_tensor_tensor(sl[:], Sc, 1.0, Lc,
                                           op0=ALU.mult, op1=ALU.mult,
                                           accum_out=slsum[:])
            # mu = SIGMA/(ns*nw) * total(SL), broadcast to all partitions
            mup = mm_ps.tile([NS, 1], F32, tag="mm_small")
            nc.tensor.matmul(mup[:], ones_sq[:], slsum[:], start=True, stop=True)
            mu = qp_tmp.tile([NS, 1], F32, tag="mu")
            nc.scalar.mul(mu[:], mup[:], SIGMA / (NS * NW))
            # r3 = SL - mu
            r3 = qp_tmp.tile([NS, NW], F32, tag="r3")
            nc.vector.tensor_scalar(r3[:], sl[:], mu[:], None, op0=ALU.subtract)
            # sinv, dv = L/s, dkd = diag(K)+1+dv, g2 = 1/dkd
            sinv = qp_tmp.tile([NS, NW], F32, tag="sinv")
            nc.vector.reciprocal(sinv[:], Sc)
            fresh, n_ns = NS_SCHED[it]
            if n_ns > 0:
                dv = qp_tmp.tile([NS, NW], F32, tag="dv")
                nc.vector.tensor_tensor(dv[:], Lc, sinv[:], op=ALU.mult)
                dkd = qp_tmp.tile([NS, NW], F32, tag="dkd")
                nc.vector.tensor_scalar(dkd[:], dv[:], dKp1[:], None,
                                        op0=ALU.add)
            if fresh:
                g2 = qp_tmp.tile([NS, NW], F32, tag="g2")
                nc.vector.reciprocal(g2[:], dkd[:])
            # r1 = GZ + Z - y1h + L + nu
            r1 = qp_tmp.tile([NS, NW], F32, tag="r1")
            nc.vector.scalar_tensor_tensor(r1[:], gzp[:], nuc, Zc,
                                           op0=ALU.add, op1=ALU.add)
            nc.vector.tensor_tensor(r1[:], r1[:], y1[:], op=ALU.subtract)
            nc.vector.tensor_tensor(r1[:], r1[:], Lc, op=ALU.add)
            # r2 = Z + S - h
            r2 = qp_tmp.tile([NS, NW], F32, tag="r2")
            nc.vector.scalar_tensor_tensor(r2[:], Zc, 1.0, Sc,
                                           op0=ALU.mult, op1=ALU.add)
            nc.vector.tensor_tensor(r2[:], r2[:], hC[:], op=ALU.subtract)
            # r4 = rowsum(Z)
            r4 = qp_tmp.tile([NS, 1], F32, tag="r4")
            nc.vector.tensor_reduce(r4[:], Zc, axis=AXL.X, op=ALU.add)
            # rhs1 = -(r1 + (L*r2 - r3)/s)
            t5 = qp_tmp.tile([NS, NW], F32, tag="t5")
            nc.vector.tensor_tensor(t5[:], Lc, r2[:], op=ALU.mult)
            nc.vector.tensor_tensor(t5[:], t5[:], r3[:], op=ALU.subtract)
            nc.vector.tensor_tensor(t5[:], t5[:], sinv[:], op=ALU.mult)
            rhs1 = qp_tmp.tile([NS, NW], F32, tag="rhs1")
            nc.vector.scalar_tensor_tensor(rhs1[:], t5[:], -1.0, r1[:],
                                           op0=ALU.mult, op1=ALU.subtract)
            # H_w = Kzd + diag(dkd_w); X0_w = diag(g2_w) on fresh iters
            for w in range(NW):
                ws = slice(w * NS, (w + 1) * NS)
                if n_ns > 0:
                    nc.vector.scalar_tensor_tensor(
                        Hs[:, ws], ident[:NS, :NS], dkd[:, w:w + 1], Kzd[:],
                        op0=ALU.mult, op1=ALU.add)
                if fresh:
                    nc.vector.tensor_scalar(X[:, ws], ident[:NS, :NS],
                                            g2[:, w:w + 1], None, op0=ALU.mult)
            # Newton-Schulz: X <- 2X - X (H X)
            for _ in range(n_ns):
                for w in range(NW):
                    ws = slice(w * NS, (w + 1) * NS)
                    yp = y_ps.tile([NS, NS], F32)
                    nc.tensor.matmul(yp[:], Hs[:, ws], X[:, ws],
                                     start=True, stop=True)
                    ysb = ns_sb.tile([NS, NS], F32, tag="ysb")
                    nc.vector.tensor_copy(ysb[:], yp[:])
                    xyp = xy_ps.tile([NS, NS], F32)
                    nc.tensor.matmul(xyp[:], X[:, ws], ysb[:],
                                     start=True, stop=True)
                    nc.vector.scalar_tensor_tensor(X[:, ws], X[:, ws], 2.0,
                                                   xyp[:], op0=ALU.mult,
                                                   op1=ALU.subtract)
            # usum = sum_w X_w rhs1_w (PSUM-accumulated across the 5 mms)
            up = mm_ps.tile([NS, 1], F32, tag="mm_small")
            for w in range(NW):
                nc.tensor.matmul(up[:], X[:, w * NS:(w + 1) * NS],
                                 rhs1[:, w:w + 1], start=(w == 0),
                                 stop=(w == NW - 1))
            # Schur = sum_w X_w ; Jacobi init for its inverse
            if n_ns > 0:
                xv = X[:].rearrange("p (w j) -> p j w", w=NW)
                nc.vector.tensor_reduce(Ssch[:], xv, axis=AXL.X, op=ALU.add)
            if fresh:
                dSg = qp_tmp.tile([NS, 1], F32, tag="dSg")
                schd = qp_tmp.tile([NS, NS], F32, tag="schd")
                nc.vector.scalar_tensor_tensor(schd[:], Ssch[:], 1.0,
                                               ident[:NS, :NS], op0=ALU.mult,
                                               op1=ALU.mult, accum_out=dSg[:])
                gs2 = qp_tmp.tile([NS, 1], F32, tag="gs2")
                nc.vector.reciprocal(gs2[:], dSg[:])
                nc.vector.tensor_scalar(XS[:], ident[:NS, :NS], gs2[:], None,
                                        op0=ALU.mult)
            for _ in range(n_ns):
                yp = y_ps.tile([NS, NS], F32)
                nc.tensor.matmul(yp[:], Ssch[:], XS[:], start=True, stop=True)
                ysb = ns_sb.tile([NS, NS], F32, tag="ysb")
                nc.scalar.copy(ysb[:], yp[:])
                xyp = xy_ps.tile([NS, NS], F32)
                nc.tensor.matmul(xyp[:], XS[:], ysb[:], start=True, stop=True)
                nc.vector.scalar_tensor_tensor(XS[:], XS[:], 2.0, xyp[:],
                                               op0=ALU.mult, op1=ALU.subtract)
            # rhs_s = usum + r4 ; dnu = XS rhs_s
            rhss = qp_tmp.tile([NS, 1], F32, tag="rhss")
            nc.vector.tensor_tensor(rhss[:], up[:], r4[:], op=ALU.add)
            dnup = mm_ps.tile([NS, 1], F32, tag="mm_small")
            nc.tensor.matmul(dnup[:], XS[:], rhss[:], start=True, stop=True)
            nc.scalar.copy(DP[:, 15:16], dnup[:])
            # dZ = X (rhs1 - dnu broadcast over w)
            rhs2 = qp_tmp.tile([NS, NW], F32, tag="rhs2")
            nc.vector.tensor_scalar(rhs2[:], rhs1[:], DP[:, 15:16], None,
                                    op0=ALU.subtract)
            xdp = mm_ps.tile([NS, NW], F32, tag="mm_small")
            for w in range(NW):
                nc.tensor.matmul(xdp[:, w:w + 1], X[:, w * NS:(w + 1) * NS],
                                 rhs2[:, w:w + 1], start=True, stop=True)
            nc.vector.tensor_copy(DP[:, 0:5], xdp[:])
            # dS = -dZ - r2 ; dL = -(L*dS + r3)/s
            nc.vector.scalar_tensor_tensor(DP[:, 5:10], DP[:, 0:5], -1.0,
                                           r2[:], op0=ALU.mult,
                                           op1=ALU.subtract)
            dl = qp_tmp.tile([NS, NW], F32, tag="dl")
            nc.vector.tensor_tensor(dl[:], Lc, DP[:, 5:10], op=ALU.mult)
            nc.vector.tensor_tensor(dl[:], dl[:], r3[:], op=ALU.add)
            nc.vector.scalar_tensor_tensor(DP[:, 10:15], dl[:], -1.0, sinv[:],
                                           op0=ALU.mult, op1=ALU.mult)
            # alpha = min(1, 0.99 / max(-d(S|L)/(S|L), eps))
            xpi = qp_tmp.tile([NS, 10], F32, tag="xpi")
            nc.vector.reciprocal(xpi[:], ST[:, 5:15])
            qr_ = qp_tmp.tile([NS, 10], F32, tag="qr_")
            nc.vector.scalar_tensor_tensor(qr_[:], DP[:, 5:15], -1.0, xpi[:],
                                           op0=ALU.mult, op1=ALU.mult)
            # global max: free-reduce, one transpose, free-reduce
            qm = qp_tmp.tile([NS, 1], F32, tag="qm")
            nc.vector.tensor_reduce(qm[:], qr_[:], axis=AXL.X, op=ALU.max)
            qtr = mm_ps.tile([1, NS], F32, tag="mm_small")
            nc.tensor.transpose(qtr[:], qm[:], ident[:NS, :NS])
            qtr_sb = qp_tmp.tile([1, NS], F32, tag="qtr_sb")
            nc.scalar.copy(qtr_sb[:], qtr[:])
            am = qp_tmp.tile([1, 1], F32, tag="am")
            nc.vector.tensor_reduce(am[:], qtr_sb[:], axis=AXL.X, op=ALU.max)
            nc.vector.tensor_scalar_max(am[:], am[:], 1e-30)
            nc.vector.reciprocal(am[:], am[:])
            nc.vector.tensor_scalar(am[:], am[:], 0.99, 1.0,
                                    op0=ALU.mult, op1=ALU.min)
            alp = mm_ps.tile([NS, 1], F32, tag="mm_small")
            nc.tensor.matmul(alp[:], ones_row[:1, :NS], am[:],
                             start=True, stop=True)
            asb = qp_tmp.tile([NS, 1], F32, tag="asb")
            nc.scalar.copy(asb[:], alp[:])
            # state += alpha * delta
            nc.vector.scalar_tensor_tensor(ST[:], DP[:], asb[:], ST[:],
                                           op0=ALU.mult, op1=ALU.add)

        # ---------------- num_sv ----------------
        cmp_ = qp_tmp.tile([NS, NW], F32, tag="cmp_")
        nc.vector.tensor_scalar(cmp_[:], ST[:, 0:5], SV_THRESH, None,
                                op0=ALU.is_gt)
        anyw = qp_tmp.tile([NS, 1], F32, tag="anyw")
        nc.vector.tensor_reduce(anyw[:], cmp_[:], axis=AXL.X, op=ALU.max)
        nsp = mm_ps.tile([NS, 1], F32, tag="mm_small")
        nc.tensor.matmul(nsp[:], ones_sq[:], anyw[:], start=True, stop=True)
        aux_sb = persist.tile([1, 4], F32)
        nc.gpsimd.memset(aux_sb[:], 0.0)
        nc.scalar.copy(aux_sb[0:1, 0:1], nsp[0:1, :])
        nc.sync.dma_start(aux_dram[:], aux_sb[:])

        # ---------------- logits = compat^T @ Z, scaled ----------------
        cp_sb = persist.tile([NS, NQC], F32)
        nc.scalar.copy(cp_sb[:], cpt[:, 0:NQC])
        for qt in range(2):
            lgp = mm_ps.tile([128, NW], F32, tag="mm_small")
            nc.tensor.matmul(lgp[:], cp_sb[:, qt * 128:(qt + 1) * 128],
                             ST[:, 0:5], start=True, stop=True)
            lg_sb = qp_tmp.tile([128, NW], F32, tag="lg_sb")
            nc.vector.tensor_scalar_mul(lg_sb[:], lgp[:], sc_bc[:])
            nc.sync.dma_start(lg_dram[qt * 128:(qt + 1) * 128, :], lg_sb[:])


_NC_CACHE = None


def _get_nc():
    global _NC_CACHE
    if _NC_CACHE is None:
        _NC_CACHE = build_nc()
    return _NC_CACHE


def _prep_in_maps(query, support, support_labels, scale):
    query = np.ascontiguousarray(np.asarray(query, np.float32))
    support = np.ascontiguousarray(np.asarray(support, np.float32))
    labels = np.asarray(support_labels).astype(np.int64)
    y1h = np.zeros((NS, NW), np.float32)
    y1h[np.arange(NS), labels] = 1.0
    sc = np.asarray(scale, np.float32).reshape(1, 1)
    return [
        {
            "qslice": query[c * NQC:(c + 1) * NQC],
            "support": support,
            "y1h": y1h,
            "scale": sc,
        }
        for c in range(N_CORES)
    ]


def run_device(query, support, support_labels, scale, trace=False, **trace_kw):
    nc = _get_nc()
    in_maps = _prep_in_maps(query, support, support_labels, scale)
    res = run_bass_kernel_spmd(nc, in_maps, list(range(N_CORES)),
                               trace=trace, **trace_kw)
    logits = np.concatenate(
        [res.results[c]["logits"] for c in range(N_CORES)], axis=0)
    num_sv = res.results[0]["aux"][0, 0]
    return logits, num_sv, res


def kernel(query, support, support_labels, n_way, n_shot, scale):
    assert int(n_way) == NW and int(n_shot) * NW == NS
    logits, num_sv, _ = run_device(query, support, support_labels, scale)
    logits_full = logits.reshape(1, NQ, NW).astype(np.float32)
    return logits_full, np.int32(round(float(num_sv)))


# revision 31
# speedup vs baseline: 1.2737x; 1.0728x over previous
"""MetaOptNet SVM-CS classification head on 8 Trainium2 NeuronCores.

Sharding: data-parallel over query rows (2048 -> 8 x 256). Each core:
  - computes the support Gram matrix K = S S^T (contraction over d=8192
    via PE-transposed support chunks), replicated;
  - runs the full interior-point QP (14 Newton steps) replicated,
    exploiting that the KKT matrix H = kron(K,I5)+I+diag(lam/s) is
    block-diagonal over the 5 classes: five 125x125 SPD solves plus a
    125x125 Schur complement for the equality constraints. All solves
    use Newton-Schulz inversion with fresh Jacobi initialization
    (rho(I - X0 H) <= ~0.3 uniformly along the trajectory);
  - streams its query slice, PE-transposes chunks, accumulates
    compat = S Q_c^T in PSUM (overlaps the QP);
  - logits_c = compat^T @ qp3, scaled by `scale`.
Host only shards inputs, concatenates the 8 logits slices, and casts
num_sv (computed on device) to int32.
"""
import sys

for _p in ("/opt/trn_rl_repo", "/root/.axon_site/_ro/trn_rl_repo"):
    if _p not in sys.path:
        sys.path.append(_p)

import numpy as np

import concourse.bacc as bacc
import concourse.mybir as mybir
import concourse.tile as tile
from concourse.bass_utils import run_bass_kernel_spmd
from concourse.masks import make_identity

F32 = mybir.dt.float32
ALU = mybir.AluOpType
AXL = mybir.AxisListType

N_CORES = 8
NQ, D, NS, NW = 2048, 8192, 125, 5
NQC = NQ // N_CORES          # 256 query rows per core
DC = D // 128                # 64 d-chunks
C_REG, SIGMA = 0.1, 0.1
# Per-iteration (reinit, newton_schulz_steps) for the H/Schur inverses:
# fresh Jacobi + 2 NS through the lam/s spike, warm-started 1 NS while it
# decays, frozen inverses for the converged tail.
NS_SCHED = [(True, 2)] * 4 + [(False, 1)] * 5 + [(False, 0)] * 3
IP_ITERS = len(NS_SCHED)
SV_THRESH = 0.001


def build_nc():
    nc = bacc.Bacc("TRN2", target_bir_lowering=False, debug=False,
                   num_devices=N_CORES)
    q_dram = nc.dram_tensor("qslice", [NQC, D], F32, kind="ExternalInput")
    s_dram = nc.dram_tensor("support", [NS, D], F32, kind="ExternalInput")
    y_dram = nc.dram_tensor("y1h", [NS, NW], F32, kind="ExternalInput")
    sc_dram = nc.dram_tensor("scale", [1, 1], F32, kind="ExternalInput")
    lg_dram = nc.dram_tensor("logits", [NQC, NW], F32, kind="ExternalOutput")
    aux_dram = nc.dram_tensor("aux", [1, 4], F32, kind="ExternalOutput")

    with tile.TileContext(nc) as tc:
        _build(tc, q_dram, s_dram, y_dram, sc_dram, lg_dram, aux_dram)
    nc.compile()
    return nc


def _build(tc, q_dram, s_dram, y_dram, sc_dram, lg_dram, aux_dram):
    nc = tc.nc
    from contextlib import ExitStack

    ctx = ExitStack()
    with ctx:
        persist = ctx.enter_context(tc.tile_pool(name="persist", bufs=1))
        qp_tmp = ctx.enter_context(tc.tile_pool(name="qp_tmp", bufs=2))
        ns_sb = ctx.enter_context(tc.tile_pool(name="ns_sb", bufs=2))
        qio = ctx.enter_context(tc.tile_pool(name="qio", bufs=3))
        qtp = ctx.enter_context(tc.tile_pool(name="qtp", bufs=3))
        tr_ps = ctx.enter_context(tc.tile_pool(name="tr_ps", bufs=2, space="PSUM"))
        mm_ps = ctx.enter_context(tc.tile_pool(name="mm_ps", bufs=1, space="PSUM"))
        y_ps = ctx.enter_context(tc.tile_pool(name="y_ps", bufs=2, space="PSUM"))
        xy_ps = ctx.enter_context(tc.tile_pool(name="xy_ps", bufs=2, space="PSUM"))
        cp_ps = ctx.enter_context(tc.tile_pool(name="cp_ps", bufs=1, space="PSUM"))

        # ---------------- constants ----------------
        ident = persist.tile([128, 128], F32)
        make_identity(nc, ident)
        ones_sq = persist.tile([NS, NS], F32)
        nc.gpsimd.memset(ones_sq[:], 1.0)
        ones_row = persist.tile([1, 128], F32)
        nc.gpsimd.memset(ones_row[:], 1.0)

        y1 = persist.tile([NS, NW], F32)
        nc.sync.dma_start(y1[:], y_dram[:])
        hC = persist.tile([NS, NW], F32)
        nc.vector.tensor_scalar_mul(hC[:], y1[:], C_REG)

        sc_sb = persist.tile([1, 1], F32)
        nc.sync.dma_start(sc_sb[:], sc_dram[:])
        scp = mm_ps.tile([128, 1], F32, tag="mm_small")
        nc.tensor.matmul(scp[:], ones_row[:], sc_sb[:], start=True, stop=True)
        sc_bc = persist.tile([128, 1], F32)
        nc.scalar.copy(sc_bc[:], scp[:])

        # compat + K share one PSUM bank: cols 0:256 compat, 256:381 K
        cpt = cp_ps.tile([NS, NQC + NS], F32)

        # ---------------- phase A: support load, S^T, K ----------------
        stq = persist.tile([128, DC * NS], F32)      # S^T chunks [128d, 125]
        with tc.tile_pool(name="snat", bufs=1) as snat_pool:
            s_nat = snat_pool.tile([NS, D], F32)
            nc.sync.dma_start(s_nat[:], s_dram[:])
            for c in range(DC):
                tp = tr_ps.tile([128, NS], F32)
                nc.tensor.transpose(tp[:], s_nat[:, c * 128:(c + 1) * 128],
                                    ident[:NS, :NS])
                if c % 2 == 0:
                    nc.scalar.copy(stq[:, c * NS:(c + 1) * NS], tp[:])
                else:
                    nc.vector.tensor_copy(stq[:, c * NS:(c + 1) * NS], tp[:])
            kp = cpt[:, NQC:NQC + NS]
            for c in range(DC):
                nc.tensor.matmul(kp, stq[:, c * NS:(c + 1) * NS],
                                 stq[:, c * NS:(c + 1) * NS],
                                 start=(c == 0), stop=(c == DC - 1))
            K_sb = persist.tile([NS, NS], F32)
            nc.scalar.copy(K_sb[:], kp)

        # Kzd = K with zeroed diagonal; diagKp1 = diag(K) + 1
        kdiag = persist.tile([NS, NS], F32)
        dK = persist.tile([NS, 1], F32)
        nc.vector.scalar_tensor_tensor(kdiag[:], K_sb[:], 1.0, ident[:NS, :NS],
                                       op0=ALU.mult, op1=ALU.mult,
                                       accum_out=dK[:])
        Kzd = persist.tile([NS, NS], F32)
        nc.vector.tensor_tensor(Kzd[:], K_sb[:], kdiag[:], op=ALU.subtract)
        dKp1 = persist.tile([NS, 1], F32)
        nc.vector.tensor_scalar_add(dKp1[:], dK[:], 1.0)

        # ---------------- phase C emit helper (independent of QP) -------
        # Both query row-blocks transposed into one [128d, 256q] rhs per
        # d-chunk -> a single compat matmul per chunk (Sᵀ weights loaded once)
        def emit_stream():
            for bc in range(4):
                qins = []
                for qt in range(2):
                    qin = qio.tile([128, 2048], F32, tag=f"qin{qt}")
                    nc.sync.dma_start(
                        qin[:], q_dram[qt * 128:(qt + 1) * 128,
                                       bc * 2048:(bc + 1) * 2048])
                    qins.append(qin)
                for sub in range(16):
                    c = bc * 16 + sub
                    qt_sb = qtp.tile([128, 256], F32)
                    for qt in range(2):
                        tp = tr_ps.tile([128, 128], F32)
                        nc.tensor.transpose(
                            tp[:], qins[qt][:, sub * 128:(sub + 1) * 128],
                            ident[:])
                        if (2 * c + qt) % 2 == 0:
                            nc.scalar.copy(qt_sb[:, qt * 128:(qt + 1) * 128],
                                           tp[:])
                        else:
                            nc.vector.tensor_copy(
                                qt_sb[:, qt * 128:(qt + 1) * 128], tp[:])
                    nc.tensor.matmul(cpt[:, 0:NQC],
                                     stq[:, c * NS:(c + 1) * NS], qt_sb[:],
                                     start=(c == 0), stop=(c == DC - 1))

        emit_stream()

        # ---------------- phase B: interior-point QP ----------------
        # state ST: cols 0:5 Z | 5:10 S | 10:15 L | 15:16 nu
        ST = persist.tile([NS, 16], F32)
        nc.gpsimd.memset(ST[:, 0:5], 0.0)
        nc.gpsimd.memset(ST[:, 5:15], 1.0)
        nc.gpsimd.memset(ST[:, 15:16], 0.0)
        DP = persist.tile([NS, 16], F32)
        X = persist.tile([NS, NW * NS], F32)         # five H_w^{-1}
        Hs = persist.tile([NS, NW * NS], F32)
        XS = persist.tile([NS, NS], F32)
        Ssch = persist.tile([NS, NS], F32)

        for it in range(IP_ITERS):
            Zc, Sc, Lc, nuc = ST[:, 0:5], ST[:, 5:10], ST[:, 10:15], ST[:, 15:16]
            # GZ = K @ Z   (K symmetric)
            gzp = mm_ps.tile([NS, NW], F32, tag="mm_small")
            nc.tensor.matmul(gzp[:], K_sb[:], Zc, start=True, stop=True)
            # SL = S*L with row sums
            sl = qp_tmp.tile([NS, NW], F32, tag="sl")
            slsum = qp_tmp.tile([NS, 1], F32, tag="slsum")
            nc.vector.scalar_tensor_tensor(sl[:], Sc, 1.0, Lc,
                                           op0=ALU.mult, op1=ALU.mult,
                                           accum_out=slsum[:])
            # mu = SIGMA/(ns*nw) * total(SL), broadcast to all partitions
            mup = mm_ps.tile([NS, 1], F32, tag="mm_small")
            nc.tensor.matmul(mup[:], ones_sq[:], slsum[:], start=True, stop=True)
            mu = qp_tmp.tile([NS, 1], F32, tag="mu")
            nc.scalar.mul(mu[:], mup[:], SIGMA / (NS * NW))
            # r3 = SL - mu
            r3 = qp_tmp.tile([NS, NW], F32, tag="r3")
            nc.vector.tensor_scalar(r3[:], sl[:], mu[:], None, op0=ALU.subtract)
            # sinv, dv = L/s, dkd = diag(K)+1+dv, g2 = 1/dkd
            sinv = qp_tmp.tile([NS, NW], F32, tag="sinv")
            nc.vector.reciprocal(sinv[:], Sc)
            fresh, n_ns = NS_SCHED[it]
            dv = qp_tmp.tile([NS, NW], F32, tag="dv")
            nc.vector.tensor_tensor(dv[:], Lc, sinv[:], op=ALU.mult)
            if n_ns > 0:
                dkd = qp_tmp.tile([NS, NW], F32, tag="dkd")
                nc.vector.tensor_scalar(dkd[:], dv[:], dKp1[:], None,
                                        op0=ALU.add)
            if fresh:
                g2 = qp_tmp.tile([NS, NW], F32, tag="g2")
                nc.vector.reciprocal(g2[:], dkd[:])
            # r1 = GZ + Z - y1h + L + nu
            r1 = qp_tmp.tile([NS, NW], F32, tag="r1")
            nc.vector.scalar_tensor_tensor(r1[:], gzp[:], nuc, Zc,
                                           op0=ALU.add, op1=ALU.add)
            nc.vector.tensor_tensor(r1[:], r1[:], y1[:], op=ALU.subtract)
            nc.vector.tensor_tensor(r1[:], r1[:], Lc, op=ALU.add)
            # r2 = Z + S - h
            r2 = qp_tmp.tile([NS, NW], F32, tag="r2")
            nc.vector.scalar_tensor_tensor(r2[:], Zc, 1.0, Sc,
                                           op0=ALU.mult, op1=ALU.add)
            nc.vector.tensor_tensor(r2[:], r2[:], hC[:], op=ALU.subtract)
            # r4 = rowsum(Z)
            r4 = qp_tmp.tile([NS, 1], F32, tag="r4")
            nc.vector.tensor_reduce(r4[:], Zc, axis=AXL.X, op=ALU.add)
            # rhs1 = -(r1 + (L*r2 - r3)/s)
            t5 = qp_tmp.tile([NS, NW], F32, tag="t5")
            nc.vector.tensor_tensor(t5[:], Lc, r2[:], op=ALU.mult)
            nc.vector.tensor_tensor(t5[:], t5[:], r3[:], op=ALU.subtract)
            nc.vector.tensor_tensor(t5[:], t5[:], sinv[:], op=ALU.mult)
            rhs1 = qp_tmp.tile([NS, NW], F32, tag="rhs1")
            nc.vector.scalar_tensor_tensor(rhs1[:], t5[:], -1.0, r1[:],
                                           op0=ALU.mult, op1=ALU.subtract)
            # H_w = Kzd + diag(dkd_w); X0_w = diag(g2_w) on fresh iters
            for w in range(NW):
                ws = slice(w * NS, (w + 1) * NS)
                if n_ns > 0:
                    nc.vector.scalar_tensor_tensor(
                        Hs[:, ws], ident[:NS, :NS], dkd[:, w:w + 1], Kzd[:],
                        op0=ALU.mult, op1=ALU.add)
                if fresh:
                    nc.vector.tensor_scalar(X[:, ws], ident[:NS, :NS],
                                            g2[:, w:w + 1], None, op0=ALU.mult)
            # Newton-Schulz: X <- 2X - X (H X)
            for _ in range(n_ns):
                for w in range(NW):
                    ws = slice(w * NS, (w + 1) * NS)
                    yp = y_ps.tile([NS, NS], F32)
                    nc.tensor.matmul(yp[:], Hs[:, ws], X[:, ws],
                                     start=True, stop=True)
                    ysb = ns_sb.tile([NS, NS], F32, tag="ysb")
                    nc.vector.tensor_copy(ysb[:], yp[:])
                    xyp = xy_ps.tile([NS, NS], F32)
                    nc.tensor.matmul(xyp[:], X[:, ws], ysb[:],
                                     start=True, stop=True)
                    nc.vector.scalar_tensor_tensor(X[:, ws], X[:, ws], 2.0,
                                                   xyp[:], op0=ALU.mult,
                                                   op1=ALU.subtract)
            # usum = sum_w X_w rhs1_w (PSUM-accumulated across the 5 mms)
            up = mm_ps.tile([NS, 1], F32, tag="mm_small")
            for w in range(NW):
                nc.tensor.matmul(up[:], X[:, w * NS:(w + 1) * NS],
                                 rhs1[:, w:w + 1], start=(w == 0),
                                 stop=(w == NW - 1))
            # Schur = sum_w X_w ; Jacobi init for its inverse
            if n_ns > 0:
                xv = X[:].rearrange("p (w j) -> p j w", w=NW)
                nc.vector.tensor_reduce(Ssch[:], xv, axis=AXL.X, op=ALU.add)
            if fresh:
                dSg = qp_tmp.tile([NS, 1], F32, tag="dSg")
                schd = qp_tmp.tile([NS, NS], F32, tag="schd")
                nc.vector.scalar_tensor_tensor(schd[:], Ssch[:], 1.0,
                                               ident[:NS, :NS], op0=ALU.mult,
                                               op1=ALU.mult, accum_out=dSg[:])
                gs2 = qp_tmp.tile([NS, 1], F32, tag="gs2")
                nc.vector.reciprocal(gs2[:], dSg[:])
                nc.vector.tensor_scalar(XS[:], ident[:NS, :NS], gs2[:], None,
                                        op0=ALU.mult)
            for _ in range(n_ns):
                yp = y_ps.tile([NS, NS], F32)
                nc.tensor.matmul(yp[:], Ssch[:], XS[:], start=True, stop=True)
                ysb = ns_sb.tile([NS, NS], F32, tag="ysb")
                nc.scalar.copy(ysb[:], yp[:])
                xyp = xy_ps.tile([NS, NS], F32)
                nc.tensor.matmul(xyp[:], XS[:], ysb[:], start=True, stop=True)
                nc.vector.scalar_tensor_tensor(XS[:], XS[:], 2.0, xyp[:],
                                               op0=ALU.mult, op1=ALU.subtract)
            # rhs_s = usum + r4 ; dnu = XS rhs_s
            rhss = qp_tmp.tile([NS, 1], F32, tag="rhss")
            nc.vector.tensor_tensor(rhss[:], up[:], r4[:], op=ALU.add)
            dnup = mm_ps.tile([NS, 1], F32, tag="mm_small")
            nc.tensor.matmul(dnup[:], XS[:], rhss[:], start=True, stop=True)
            nc.scalar.copy(DP[:, 15:16], dnup[:])
            # dZ = X (rhs1 - dnu broadcast over w)
            rhs2 = qp_tmp.tile([NS, NW], F32, tag="rhs2")
            nc.vector.tensor_scalar(rhs2[:], rhs1[:], DP[:, 15:16], None,
                                    op0=ALU.subtract)
            xdp = mm_ps.tile([NS, NW], F32, tag="mm_small")
            for w in range(NW):
                nc.tensor.matmul(xdp[:, w:w + 1], X[:, w * NS:(w + 1) * NS],
                                 rhs2[:, w:w + 1], start=True, stop=True)
            nc.vector.tensor_copy(DP[:, 0:5], xdp[:])
            # dS = -dZ - r2 ; dL = dZ*(L/s) + (L*r2 - r3)/s  (reuses t5)
            nc.vector.scalar_tensor_tensor(DP[:, 5:10], DP[:, 0:5], -1.0,
                                           r2[:], op0=ALU.mult,
                                           op1=ALU.subtract)
            dl = qp_tmp.tile([NS, NW], F32, tag="dl")
            nc.vector.tensor_tensor(dl[:], DP[:, 0:5], dv[:], op=ALU.mult)
            nc.vector.tensor_tensor(DP[:, 10:15], dl[:], t5[:], op=ALU.add)
            # alpha = min(1, 0.99 / max(-d(S|L)/(S|L), eps))
            xpi = qp_tmp.tile([NS, 10], F32, tag="xpi")
            nc.vector.reciprocal(xpi[:], ST[:, 5:15])
            qr_ = qp_tmp.tile([NS, 10], F32, tag="qr_")
            nc.vector.scalar_tensor_tensor(qr_[:], DP[:, 5:15], -1.0, xpi[:],
                                           op0=ALU.mult, op1=ALU.mult)
            # global max: free-reduce, one transpose, free-reduce
            qm = qp_tmp.tile([NS, 1], F32, tag="qm")
            nc.vector.tensor_reduce(qm[:], qr_[:], axis=AXL.X, op=ALU.max)
            qtr = mm_ps.tile([1, NS], F32, tag="mm_small")
            nc.tensor.transpose(qtr[:], qm[:], ident[:NS, :NS])
            qtr_sb = qp_tmp.tile([1, NS], F32, tag="qtr_sb")
            nc.scalar.copy(qtr_sb[:], qtr[:])
            am = qp_tmp.tile([1, 1], F32, tag="am")
            nc.vector.tensor_reduce(am[:], qtr_sb[:], axis=AXL.X, op=ALU.max)
            nc.vector.tensor_scalar_max(am[:], am[:], 1e-30)
            nc.vector.reciprocal(am[:], am[:])
            nc.vector.tensor_scalar(am[:], am[:], 0.99, 1.0,
                                    op0=ALU.mult, op1=ALU.min)
            alp = mm_ps.tile([NS, 1], F32, tag="mm_small")
            nc.tensor.matmul(alp[:], ones_row[:1, :NS], am[:],
                             start=True, stop=True)
            asb = qp_tmp.tile([NS, 1], F32, tag="asb")
            nc.scalar.copy(asb[:], alp[:])
            # state += alpha * delta
            nc.vector.scalar_tensor_tensor(ST[:], DP[:], asb[:], ST[:],
                                           op0=ALU.mult, op1=ALU.add)

        # ---------------- num_sv ----------------
        cmp_ = qp_tmp.tile([NS, NW], F32, tag="cmp_")
        nc.vector.tensor_scalar(cmp_[:], ST[:, 0:5], SV_THRESH, None,
                                op0=ALU.is_gt)
        anyw = qp_tmp.tile([NS, 1], F32, tag="anyw")
        nc.vector.tensor_reduce(anyw[:], cmp_[:], axis=AXL.X, op=ALU.max)
        nsp = mm_ps.tile([NS, 1], F32, tag="mm_small")
        nc.tensor.matmul(nsp[:], ones_sq[:], anyw[:], start=True, stop=True)
        aux_sb = persist.tile([1, 4], F32)
        nc.gpsimd.memset(aux_sb[:], 0.0)
        nc.scalar.copy(aux_sb[0:1, 0:1], nsp[0:1, :])
        nc.sync.dma_start(aux_dram[:], aux_sb[:])

        # ---------------- logits = compat^T @ Z, scaled ----------------
        cp_sb = persist.tile([NS, NQC], F32)
        nc.scalar.copy(cp_sb[:], cpt[:, 0:NQC])
        for qt in range(2):
            lgp = mm_ps.tile([128, NW], F32, tag="mm_small")
            nc.tensor.matmul(lgp[:], cp_sb[:, qt * 128:(qt + 1) * 128],
                             ST[:, 0:5], start=True, stop=True)
            lg_sb = qp_tmp.tile([128, NW], F32, tag="lg_sb")
            nc.vector.tensor_scalar_mul(lg_sb[:], lgp[:], sc_bc[:])
            nc.sync.dma_start(lg_dram[qt * 128:(qt + 1) * 128, :], lg_sb[:])


_NC_CACHE = None


def _get_nc():
    global _NC_CACHE
    if _NC_CACHE is None:
        _NC_CACHE = build_nc()
    return _NC_CACHE


def _prep_in_maps(query, support, support_labels, scale):
    query = np.ascontiguousarray(np.asarray(query, np.float32))
    support = np.ascontiguousarray(np.asarray(support, np.float32))
    labels = np.asarray(support_labels).astype(np.int64)
    y1h = np.zeros((NS, NW), np.float32)
    y1h[np.arange(NS), labels] = 1.0
    sc = np.asarray(scale, np.float32).reshape(1, 1)
    return [
        {
            "qslice": query[c * NQC:(c + 1) * NQC],
            "support": support,
            "y1h": y1h,
            "scale": sc,
        }
        for c in range(N_CORES)
    ]


def run_device(query, support, support_labels, scale, trace=False, **trace_kw):
    nc = _get_nc()
    in_maps = _prep_in_maps(query, support, support_labels, scale)
    res = run_bass_kernel_spmd(nc, in_maps, list(range(N_CORES)),
                               trace=trace, **trace_kw)
    logits = np.concatenate(
        [res.results[c]["logits"] for c in range(N_CORES)], axis=0)
    num_sv = res.results[0]["aux"][0, 0]
    return logits, num_sv, res


def kernel(query, support, support_labels, n_way, n_shot, scale):
    assert int(n_way) == NW and int(n_shot) * NW == NS
    logits, num_sv, _ = run_device(query, support, support_labels, scale)
    logits_full = logits.reshape(1, NQ, NW).astype(np.float32)
    return logits_full, np.int32(round(float(num_sv)))


# revision 32
# speedup vs baseline: 1.2975x; 1.0186x over previous
"""MetaOptNet SVM-CS classification head on 8 Trainium2 NeuronCores.

Sharding: data-parallel over query rows (2048 -> 8 x 256). Each core:
  - computes the support Gram matrix K = S S^T (contraction over d=8192
    via PE-transposed support chunks), replicated;
  - runs the full interior-point QP (14 Newton steps) replicated,
    exploiting that the KKT matrix H = kron(K,I5)+I+diag(lam/s) is
    block-diagonal over the 5 classes: five 125x125 SPD solves plus a
    125x125 Schur complement for the equality constraints. All solves
    use Newton-Schulz inversion with fresh Jacobi initialization
    (rho(I - X0 H) <= ~0.3 uniformly along the trajectory);
  - streams its query slice, PE-transposes chunks, accumulates
    compat = S Q_c^T in PSUM (overlaps the QP);
  - logits_c = compat^T @ qp3, scaled by `scale`.
Host only shards inputs, concatenates the 8 logits slices, and casts
num_sv (computed on device) to int32.
"""
import sys

for _p in ("/opt/trn_rl_repo", "/root/.axon_site/_ro/trn_rl_repo"):
    if _p not in sys.path:
        sys.path.append(_p)

import numpy as np

import concourse.bacc as bacc
import concourse.mybir as mybir
import concourse.tile as tile
from concourse.bass_utils import run_bass_kernel_spmd
from concourse.masks import make_identity

F32 = mybir.dt.float32
ALU = mybir.AluOpType
AXL = mybir.AxisListType

N_CORES = 8
NQ, D, NS, NW = 2048, 8192, 125, 5
NQC = NQ // N_CORES          # 256 query rows per core
DC = D // 128                # 64 d-chunks
C_REG, SIGMA = 0.1, 0.1
# Per-iteration (reinit, newton_schulz_steps) for the H/Schur inverses:
# fresh Jacobi + 2 NS through the lam/s spike, warm-started 1 NS while it
# decays, frozen inverses for the converged tail.
NS_SCHED = [(True, 2)] * 4 + [(False, 1)] * 4 + [(False, 0)] * 4
IP_ITERS = len(NS_SCHED)
SV_THRESH = 0.001


def build_nc():
    nc = bacc.Bacc("TRN2", target_bir_lowering=False, debug=False,
                   num_devices=N_CORES)
    q_dram = nc.dram_tensor("qslice", [NQC, D], F32, kind="ExternalInput")
    s_dram = nc.dram_tensor("support", [NS, D], F32, kind="ExternalInput")
    y_dram = nc.dram_tensor("y1h", [NS, NW], F32, kind="ExternalInput")
    sc_dram = nc.dram_tensor("scale", [1, 1], F32, kind="ExternalInput")
    lg_dram = nc.dram_tensor("logits", [NQC, NW], F32, kind="ExternalOutput")
    aux_dram = nc.dram_tensor("aux", [1, 4], F32, kind="ExternalOutput")

    with tile.TileContext(nc) as tc:
        _build(tc, q_dram, s_dram, y_dram, sc_dram, lg_dram, aux_dram)
    nc.compile()
    return nc


def _build(tc, q_dram, s_dram, y_dram, sc_dram, lg_dram, aux_dram):
    nc = tc.nc
    from contextlib import ExitStack

    ctx = ExitStack()
    with ctx:
        persist = ctx.enter_context(tc.tile_pool(name="persist", bufs=1))
        qp_tmp = ctx.enter_context(tc.tile_pool(name="qp_tmp", bufs=2))
        ns_sb = ctx.enter_context(tc.tile_pool(name="ns_sb", bufs=2))
        qio = ctx.enter_context(tc.tile_pool(name="qio", bufs=3))
        qtp = ctx.enter_context(tc.tile_pool(name="qtp", bufs=3))
        tr_ps = ctx.enter_context(tc.tile_pool(name="tr_ps", bufs=2, space="PSUM"))
        mm_ps = ctx.enter_context(tc.tile_pool(name="mm_ps", bufs=1, space="PSUM"))
        y_ps = ctx.enter_context(tc.tile_pool(name="y_ps", bufs=2, space="PSUM"))
        xy_ps = ctx.enter_context(tc.tile_pool(name="xy_ps", bufs=2, space="PSUM"))
        cp_ps = ctx.enter_context(tc.tile_pool(name="cp_ps", bufs=1, space="PSUM"))

        # ---------------- constants ----------------
        ident = persist.tile([128, 128], F32)
        make_identity(nc, ident)
        ones_sq = persist.tile([NS, NS], F32)
        nc.gpsimd.memset(ones_sq[:], 1.0)
        ones_row = persist.tile([1, 128], F32)
        nc.gpsimd.memset(ones_row[:], 1.0)

        y1 = persist.tile([NS, NW], F32)
        nc.sync.dma_start(y1[:], y_dram[:])
        hC = persist.tile([NS, NW], F32)
        nc.vector.tensor_scalar_mul(hC[:], y1[:], C_REG)

        sc_sb = persist.tile([1, 1], F32)
        nc.sync.dma_start(sc_sb[:], sc_dram[:])
        scp = mm_ps.tile([128, 1], F32, tag="mm_small")
        nc.tensor.matmul(scp[:], ones_row[:], sc_sb[:], start=True, stop=True)
        sc_bc = persist.tile([128, 1], F32)
        nc.scalar.copy(sc_bc[:], scp[:])

        # compat + K share one PSUM bank: cols 0:256 compat, 256:381 K
        cpt = cp_ps.tile([NS, NQC + NS], F32)

        # ---------------- phase A: support load, S^T, K ----------------
        stq = persist.tile([128, DC * NS], F32)      # S^T chunks [128d, 125]
        with tc.tile_pool(name="snat", bufs=1) as snat_pool:
            s_nat = snat_pool.tile([NS, D], F32)
            nc.sync.dma_start(s_nat[:], s_dram[:])
            for c in range(DC):
                tp = tr_ps.tile([128, NS], F32)
                nc.tensor.transpose(tp[:], s_nat[:, c * 128:(c + 1) * 128],
                                    ident[:NS, :NS])
                if c % 2 == 0:
                    nc.scalar.copy(stq[:, c * NS:(c + 1) * NS], tp[:])
                else:
                    nc.vector.tensor_copy(stq[:, c * NS:(c + 1) * NS], tp[:])
            kp = cpt[:, NQC:NQC + NS]
            for c in range(DC):
                nc.tensor.matmul(kp, stq[:, c * NS:(c + 1) * NS],
                                 stq[:, c * NS:(c + 1) * NS],
                                 start=(c == 0), stop=(c == DC - 1))
            K_sb = persist.tile([NS, NS], F32)
            nc.scalar.copy(K_sb[:], kp)

        # Kzd = K with zeroed diagonal; diagKp1 = diag(K) + 1
        kdiag = persist.tile([NS, NS], F32)
        dK = persist.tile([NS, 1], F32)
        nc.vector.scalar_tensor_tensor(kdiag[:], K_sb[:], 1.0, ident[:NS, :NS],
                                       op0=ALU.mult, op1=ALU.mult,
                                       accum_out=dK[:])
        Kzd = persist.tile([NS, NS], F32)
        nc.vector.tensor_tensor(Kzd[:], K_sb[:], kdiag[:], op=ALU.subtract)
        dKp1 = persist.tile([NS, 1], F32)
        nc.vector.tensor_scalar_add(dKp1[:], dK[:], 1.0)

        # ---------------- phase C emit helper (independent of QP) -------
        # Both query row-blocks transposed into one [128d, 256q] rhs per
        # d-chunk -> a single compat matmul per chunk (Sᵀ weights loaded once)
        def emit_stream():
            for bc in range(4):
                qins = []
                for qt in range(2):
                    qin = qio.tile([128, 2048], F32, tag=f"qin{qt}")
                    nc.sync.dma_start(
                        qin[:], q_dram[qt * 128:(qt + 1) * 128,
                                       bc * 2048:(bc + 1) * 2048])
                    qins.append(qin)
                for sub in range(16):
                    c = bc * 16 + sub
                    qt_sb = qtp.tile([128, 256], F32)
                    for qt in range(2):
                        tp = tr_ps.tile([128, 128], F32)
                        nc.tensor.transpose(
                            tp[:], qins[qt][:, sub * 128:(sub + 1) * 128],
                            ident[:])
                        if (2 * c + qt) % 2 == 0:
                            nc.scalar.copy(qt_sb[:, qt * 128:(qt + 1) * 128],
                                           tp[:])
                        else:
                            nc.vector.tensor_copy(
                                qt_sb[:, qt * 128:(qt + 1) * 128], tp[:])
                    nc.tensor.matmul(cpt[:, 0:NQC],
                                     stq[:, c * NS:(c + 1) * NS], qt_sb[:],
                                     start=(c == 0), stop=(c == DC - 1))

        emit_stream()

        # ---------------- phase B: interior-point QP ----------------
        # state ST: cols 0:5 Z | 5:10 S | 10:15 L | 15:16 nu
        ST = persist.tile([NS, 16], F32)
        nc.gpsimd.memset(ST[:, 0:5], 0.0)
        nc.gpsimd.memset(ST[:, 5:15], 1.0)
        nc.gpsimd.memset(ST[:, 15:16], 0.0)
        DP = persist.tile([NS, 16], F32)
        X = persist.tile([NS, NW * NS], F32)         # five H_w^{-1}
        Hs = persist.tile([NS, NW * NS], F32)
        XS = persist.tile([NS, NS], F32)
        Ssch = persist.tile([NS, NS], F32)

        for it in range(IP_ITERS):
            Zc, Sc, Lc, nuc = ST[:, 0:5], ST[:, 5:10], ST[:, 10:15], ST[:, 15:16]
            # GZ = K @ Z   (K symmetric)
            gzp = mm_ps.tile([NS, NW], F32, tag="mm_small")
            nc.tensor.matmul(gzp[:], K_sb[:], Zc, start=True, stop=True)
            # SL = S*L with row sums
            sl = qp_tmp.tile([NS, NW], F32, tag="sl")
            slsum = qp_tmp.tile([NS, 1], F32, tag="slsum")
            nc.vector.scalar_tensor_tensor(sl[:], Sc, 1.0, Lc,
                                           op0=ALU.mult, op1=ALU.mult,
                                           accum_out=slsum[:])
            # mu = SIGMA/(ns*nw) * total(SL), broadcast to all partitions
            mup = mm_ps.tile([NS, 1], F32, tag="mm_small")
            nc.tensor.matmul(mup[:], ones_sq[:], slsum[:], start=True, stop=True)
            mu = qp_tmp.tile([NS, 1], F32, tag="mu")
            nc.scalar.mul(mu[:], mup[:], SIGMA / (NS * NW))
            # r3 = SL - mu
            r3 = qp_tmp.tile([NS, NW], F32, tag="r3")
            nc.vector.tensor_scalar(r3[:], sl[:], mu[:], None, op0=ALU.subtract)
            # sinv, dv = L/s, dkd = diag(K)+1+dv, g2 = 1/dkd
            sinv = qp_tmp.tile([NS, NW], F32, tag="sinv")
            nc.vector.reciprocal(sinv[:], Sc)
            fresh, n_ns = NS_SCHED[it]
            dv = qp_tmp.tile([NS, NW], F32, tag="dv")
            nc.vector.tensor_tensor(dv[:], Lc, sinv[:], op=ALU.mult)
            if n_ns > 0:
                dkd = qp_tmp.tile([NS, NW], F32, tag="dkd")
                nc.vector.tensor_scalar(dkd[:], dv[:], dKp1[:], None,
                                        op0=ALU.add)
            if fresh:
                g2 = qp_tmp.tile([NS, NW], F32, tag="g2")
                nc.vector.reciprocal(g2[:], dkd[:])
            # r1 = GZ + Z - y1h + L + nu
            r1 = qp_tmp.tile([NS, NW], F32, tag="r1")
            nc.vector.scalar_tensor_tensor(r1[:], gzp[:], nuc, Zc,
                                           op0=ALU.add, op1=ALU.add)
            nc.vector.tensor_tensor(r1[:], r1[:], y1[:], op=ALU.subtract)
            nc.vector.tensor_tensor(r1[:], r1[:], Lc, op=ALU.add)
            # r2 = Z + S - h
            r2 = qp_tmp.tile([NS, NW], F32, tag="r2")
            nc.vector.scalar_tensor_tensor(r2[:], Zc, 1.0, Sc,
                                           op0=ALU.mult, op1=ALU.add)
            nc.vector.tensor_tensor(r2[:], r2[:], hC[:], op=ALU.subtract)
            # r4 = rowsum(Z)
            r4 = qp_tmp.tile([NS, 1], F32, tag="r4")
            nc.vector.tensor_reduce(r4[:], Zc, axis=AXL.X, op=ALU.add)
            # rhs1 = -(r1 + (L*r2 - r3)/s)
            t5 = qp_tmp.tile([NS, NW], F32, tag="t5")
            nc.vector.tensor_tensor(t5[:], Lc, r2[:], op=ALU.mult)
            nc.vector.tensor_tensor(t5[:], t5[:], r3[:], op=ALU.subtract)
            nc.vector.tensor_tensor(t5[:], t5[:], sinv[:], op=ALU.mult)
            rhs1 = qp_tmp.tile([NS, NW], F32, tag="rhs1")
            nc.vector.scalar_tensor_tensor(rhs1[:], t5[:], -1.0, r1[:],
                                           op0=ALU.mult, op1=ALU.subtract)
            # H_w = Kzd + diag(dkd_w); X0_w = diag(g2_w) on fresh iters
            for w in range(NW):
                ws = slice(w * NS, (w + 1) * NS)
                if n_ns > 0:
                    nc.vector.scalar_tensor_tensor(
                        Hs[:, ws], ident[:NS, :NS], dkd[:, w:w + 1], Kzd[:],
                        op0=ALU.mult, op1=ALU.add)
                if fresh:
                    nc.vector.tensor_scalar(X[:, ws], ident[:NS, :NS],
                                            g2[:, w:w + 1], None, op0=ALU.mult)
            # Newton-Schulz: X <- 2X - X (H X)
            for _ in range(n_ns):
                for w in range(NW):
                    ws = slice(w * NS, (w + 1) * NS)
                    yp = y_ps.tile([NS, NS], F32)
                    nc.tensor.matmul(yp[:], Hs[:, ws], X[:, ws],
                                     start=True, stop=True)
                    ysb = ns_sb.tile([NS, NS], F32, tag="ysb")
                    nc.vector.tensor_copy(ysb[:], yp[:])
                    xyp = xy_ps.tile([NS, NS], F32)
                    nc.tensor.matmul(xyp[:], X[:, ws], ysb[:],
                                     start=True, stop=True)
                    nc.vector.scalar_tensor_tensor(X[:, ws], X[:, ws], 2.0,
                                                   xyp[:], op0=ALU.mult,
                                                   op1=ALU.subtract)
            # usum = sum_w X_w rhs1_w (PSUM-accumulated across the 5 mms)
            up = mm_ps.tile([NS, 1], F32, tag="mm_small")
            for w in range(NW):
                nc.tensor.matmul(up[:], X[:, w * NS:(w + 1) * NS],
                                 rhs1[:, w:w + 1], start=(w == 0),
                                 stop=(w == NW - 1))
            # Schur = sum_w X_w ; Jacobi init for its inverse
            if n_ns > 0:
                xv = X[:].rearrange("p (w j) -> p j w", w=NW)
                nc.vector.tensor_reduce(Ssch[:], xv, axis=AXL.X, op=ALU.add)
            if fresh:
                dSg = qp_tmp.tile([NS, 1], F32, tag="dSg")
                schd = qp_tmp.tile([NS, NS], F32, tag="schd")
                nc.vector.scalar_tensor_tensor(schd[:], Ssch[:], 1.0,
                                               ident[:NS, :NS], op0=ALU.mult,
                                               op1=ALU.mult, accum_out=dSg[:])
                gs2 = qp_tmp.tile([NS, 1], F32, tag="gs2")
                nc.vector.reciprocal(gs2[:], dSg[:])
                nc.vector.tensor_scalar(XS[:], ident[:NS, :NS], gs2[:], None,
                                        op0=ALU.mult)
            for _ in range(n_ns):
                yp = y_ps.tile([NS, NS], F32)
                nc.tensor.matmul(yp[:], Ssch[:], XS[:], start=True, stop=True)
                ysb = ns_sb.tile([NS, NS], F32, tag="ysb")
                nc.scalar.copy(ysb[:], yp[:])
                xyp = xy_ps.tile([NS, NS], F32)
                nc.tensor.matmul(xyp[:], XS[:], ysb[:], start=True, stop=True)
                nc.vector.scalar_tensor_tensor(XS[:], XS[:], 2.0, xyp[:],
                                               op0=ALU.mult, op1=ALU.subtract)
            # rhs_s = usum + r4 ; dnu = XS rhs_s
            rhss = qp_tmp.tile([NS, 1], F32, tag="rhss")
            nc.vector.tensor_tensor(rhss[:], up[:], r4[:], op=ALU.add)
            dnup = mm_ps.tile([NS, 1], F32, tag="mm_small")
            nc.tensor.matmul(dnup[:], XS[:], rhss[:], start=True, stop=True)
            nc.scalar.copy(DP[:, 15:16], dnup[:])
            # dZ = X (rhs1 - dnu broadcast over w)
            rhs2 = qp_tmp.tile([NS, NW], F32, tag="rhs2")
            nc.vector.tensor_scalar(rhs2[:], rhs1[:], DP[:, 15:16], None,
                                    op0=ALU.subtract)
            xdp = mm_ps.tile([NS, NW], F32, tag="mm_small")
            for w in range(NW):
                nc.tensor.matmul(xdp[:, w:w + 1], X[:, w * NS:(w + 1) * NS],
                                 rhs2[:, w:w + 1], start=True, stop=True)
            nc.vector.tensor_copy(DP[:, 0:5], xdp[:])
            # dS = -dZ - r2 ; dL = dZ*(L/s) + (L*r2 - r3)/s  (reuses t5)
            nc.vector.scalar_tensor_tensor(DP[:, 5:10], DP[:, 0:5], -1.0,
                                           r2[:], op0=ALU.mult,
                                           op1=ALU.subtract)
            dl = qp_tmp.tile([NS, NW], F32, tag="dl")
            nc.vector.tensor_tensor(dl[:], DP[:, 0:5], dv[:], op=ALU.mult)
            nc.vector.tensor_tensor(DP[:, 10:15], dl[:], t5[:], op=ALU.add)
            # alpha = min(1, 0.99 / max(-d(S|L)/(S|L), eps))
            xpi = qp_tmp.tile([NS, 10], F32, tag="xpi")
            nc.vector.reciprocal(xpi[:], ST[:, 5:15])
            qr_ = qp_tmp.tile([NS, 10], F32, tag="qr_")
            nc.vector.scalar_tensor_tensor(qr_[:], DP[:, 5:15], -1.0, xpi[:],
                                           op0=ALU.mult, op1=ALU.mult)
            # global max: free-reduce, one transpose, free-reduce
            qm = qp_tmp.tile([NS, 1], F32, tag="qm")
            nc.vector.tensor_reduce(qm[:], qr_[:], axis=AXL.X, op=ALU.max)
            qtr = mm_ps.tile([1, NS], F32, tag="mm_small")
            nc.tensor.transpose(qtr[:], qm[:], ident[:NS, :NS])
            qtr_sb = qp_tmp.tile([1, NS], F32, tag="qtr_sb")
            nc.scalar.copy(qtr_sb[:], qtr[:])
            am = qp_tmp.tile([1, 1], F32, tag="am")
            nc.vector.tensor_reduce(am[:], qtr_sb[:], axis=AXL.X, op=ALU.max)
            nc.vector.tensor_scalar_max(am[:], am[:], 1e-30)
            nc.vector.reciprocal(am[:], am[:])
            nc.vector.tensor_scalar(am[:], am[:], 0.99, 1.0,
                                    op0=ALU.mult, op1=ALU.min)
            alp = mm_ps.tile([NS, 1], F32, tag="mm_small")
            nc.tensor.matmul(alp[:], ones_row[:1, :NS], am[:],
                             start=True, stop=True)
            asb = qp_tmp.tile([NS, 1], F32, tag="asb")
            nc.scalar.copy(asb[:], alp[:])
            # state += alpha * delta
            nc.vector.scalar_tensor_tensor(ST[:], DP[:], asb[:], ST[:],
                                           op0=ALU.mult, op1=ALU.add)

        # ---------------- num_sv ----------------
        cmp_ = qp_tmp.tile([NS, NW], F32, tag="cmp_")
        nc.vector.tensor_scalar(cmp_[:], ST[:, 0:5], SV_THRESH, None,
                                op0=ALU.is_gt)
        anyw = qp_tmp.tile([NS, 1], F32, tag="anyw")
        nc.vector.tensor_reduce(anyw[:], cmp_[:], axis=AXL.X, op=ALU.max)
        nsp = mm_ps.tile([NS, 1], F32, tag="mm_small")
        nc.tensor.matmul(nsp[:], ones_sq[:], anyw[:], start=True, stop=True)
        aux_sb = persist.tile([1, 4], F32)
        nc.gpsimd.memset(aux_sb[:], 0.0)
        nc.scalar.copy(aux_sb[0:1, 0:1], nsp[0:1, :])
        nc.sync.dma_start(aux_dram[:], aux_sb[:])

        # ---------------- logits = compat^T @ Z, scaled ----------------
        cp_sb = persist.tile([NS, NQC], F32)
        nc.scalar.copy(cp_sb[:], cpt[:, 0:NQC])
        for qt in range(2):
            lgp = mm_ps.tile([128, NW], F32, tag="mm_small")
            nc.tensor.matmul(lgp[:], cp_sb[:, qt * 128:(qt + 1) * 128],
                             ST[:, 0:5], start=True, stop=True)
            lg_sb = qp_tmp.tile([128, NW], F32, tag="lg_sb")
            nc.vector.tensor_scalar_mul(lg_sb[:], lgp[:], sc_bc[:])
            nc.sync.dma_start(lg_dram[qt * 128:(qt + 1) * 128, :], lg_sb[:])


_NC_CACHE = None


def _get_nc():
    global _NC_CACHE
    if _NC_CACHE is None:
        _NC_CACHE = build_nc()
    return _NC_CACHE


def _prep_in_maps(query, support, support_labels, scale):
    query = np.ascontiguousarray(np.asarray(query, np.float32))
    support = np.ascontiguousarray(np.asarray(support, np.float32))
    labels = np.asarray(support_labels).astype(np.int64)
    y1h = np.zeros((NS, NW), np.float32)
    y1h[np.arange(NS), labels] = 1.0
    sc = np.asarray(scale, np.float32).reshape(1, 1)
    return [
        {
            "qslice": query[c * NQC:(c + 1) * NQC],
            "support": support,
            "y1h": y1h,
            "scale": sc,
        }
        for c in range(N_CORES)
    ]


def run_device(query, support, support_labels, scale, trace=False, **trace_kw):
    nc = _get_nc()
    in_maps = _prep_in_maps(query, support, support_labels, scale)
    res = run_bass_kernel_spmd(nc, in_maps, list(range(N_CORES)),
                               trace=trace, **trace_kw)
    logits = np.concatenate(
        [res.results[c]["logits"] for c in range(N_CORES)], axis=0)
    num_sv = res.results[0]["aux"][0, 0]
    return logits, num_sv, res


def kernel(query, support, support_labels, n_way, n_shot, scale):
    assert int(n_way) == NW and int(n_shot) * NW == NS
    logits, num_sv, _ = run_device(query, support, support_labels, scale)
    logits_full = logits.reshape(1, NQ, NW).astype(np.float32)
    return logits_full, np.int32(round(float(num_sv)))


# revision 34
# speedup vs baseline: 1.3980x; 1.0775x over previous
"""MetaOptNet SVM-CS classification head on 8 Trainium2 NeuronCores.

Sharding: data-parallel over query rows (2048 -> 8 x 256). Each core:
  - computes the support Gram matrix K = S S^T (contraction over d=8192
    via PE-transposed support chunks), replicated;
  - runs the full interior-point QP (14 Newton steps) replicated,
    exploiting that the KKT matrix H = kron(K,I5)+I+diag(lam/s) is
    block-diagonal over the 5 classes: five 125x125 SPD solves plus a
    125x125 Schur complement for the equality constraints. All solves
    use Newton-Schulz inversion with fresh Jacobi initialization
    (rho(I - X0 H) <= ~0.3 uniformly along the trajectory);
  - streams its query slice, PE-transposes chunks, accumulates
    compat = S Q_c^T in PSUM (overlaps the QP);
  - logits_c = compat^T @ qp3, scaled by `scale`.
Host only shards inputs, concatenates the 8 logits slices, and casts
num_sv (computed on device) to int32.
"""
import sys

for _p in ("/opt/trn_rl_repo", "/root/.axon_site/_ro/trn_rl_repo"):
    if _p not in sys.path:
        sys.path.append(_p)

import numpy as np

import concourse.bacc as bacc
import concourse.mybir as mybir
import concourse.tile as tile
from concourse.bass_utils import run_bass_kernel_spmd
from concourse.masks import make_identity

F32 = mybir.dt.float32
ALU = mybir.AluOpType
AXL = mybir.AxisListType

N_CORES = 8
NQ, D, NS, NW = 2048, 8192, 125, 5
NQC = NQ // N_CORES          # 256 query rows per core
DC = D // 128                # 64 d-chunks
C_REG, SIGMA = 0.1, 0.1
# Per-iteration (reinit, newton_schulz_steps) for the H/Schur inverses:
# fresh Jacobi + 2 NS through the lam/s spike, warm-started 1 NS while it
# decays, frozen inverses for the converged tail.
NS_SCHED = [(True, 2)] * 4 + [(False, 1)] * 5 + [(False, 0)] * 3
IP_ITERS = len(NS_SCHED)
SV_THRESH = 0.001


def build_nc():
    nc = bacc.Bacc("TRN2", target_bir_lowering=False, debug=False,
                   num_devices=N_CORES)
    q_dram = nc.dram_tensor("qslice", [NQC, D], F32, kind="ExternalInput")
    s_dram = nc.dram_tensor("support", [NS, D], F32, kind="ExternalInput")
    y_dram = nc.dram_tensor("y1h", [NS, NW], F32, kind="ExternalInput")
    sc_dram = nc.dram_tensor("scale", [1, 1], F32, kind="ExternalInput")
    lg_dram = nc.dram_tensor("logits", [NQC, NW], F32, kind="ExternalOutput")
    aux_dram = nc.dram_tensor("aux", [1, 4], F32, kind="ExternalOutput")

    with tile.TileContext(nc) as tc:
        _build(tc, q_dram, s_dram, y_dram, sc_dram, lg_dram, aux_dram)
    nc.compile()
    return nc


def _build(tc, q_dram, s_dram, y_dram, sc_dram, lg_dram, aux_dram):
    nc = tc.nc
    from contextlib import ExitStack

    ctx = ExitStack()
    with ctx:
        persist = ctx.enter_context(tc.tile_pool(name="persist", bufs=1))
        qp_tmp = ctx.enter_context(tc.tile_pool(name="qp_tmp", bufs=2))
        ns_sb = ctx.enter_context(tc.tile_pool(name="ns_sb", bufs=2))
        qio = ctx.enter_context(tc.tile_pool(name="qio", bufs=3))
        qtp = ctx.enter_context(tc.tile_pool(name="qtp", bufs=3))
        tr_ps = ctx.enter_context(tc.tile_pool(name="tr_ps", bufs=2, space="PSUM"))
        mm_ps = ctx.enter_context(tc.tile_pool(name="mm_ps", bufs=1, space="PSUM"))
        y_ps = ctx.enter_context(tc.tile_pool(name="y_ps", bufs=2, space="PSUM"))
        xy_ps = ctx.enter_context(tc.tile_pool(name="xy_ps", bufs=2, space="PSUM"))
        cp_ps = ctx.enter_context(tc.tile_pool(name="cp_ps", bufs=1, space="PSUM"))

        # ---------------- constants ----------------
        ident = persist.tile([128, 128], F32)
        make_identity(nc, ident)
        ones_sq = persist.tile([NS, NS], F32)
        nc.gpsimd.memset(ones_sq[:], 1.0)
        ones_row = persist.tile([1, 128], F32)
        nc.gpsimd.memset(ones_row[:], 1.0)

        y1 = persist.tile([NS, NW], F32)
        nc.sync.dma_start(y1[:], y_dram[:])
        hC = persist.tile([NS, NW], F32)
        nc.vector.tensor_scalar_mul(hC[:], y1[:], C_REG)

        sc_sb = persist.tile([1, 1], F32)
        nc.sync.dma_start(sc_sb[:], sc_dram[:])
        scp = mm_ps.tile([128, 1], F32, tag="mm_small")
        nc.tensor.matmul(scp[:], ones_row[:], sc_sb[:], start=True, stop=True)
        sc_bc = persist.tile([128, 1], F32)
        nc.scalar.copy(sc_bc[:], scp[:])

        # compat + K share one PSUM bank: cols 0:256 compat, 256:381 K
        cpt = cp_ps.tile([NS, NQC + NS], F32)

        # ---------------- phase A: support load, S^T, K ----------------
        stq = persist.tile([128, DC * NS], F32)      # S^T chunks [128d, 125]
        with tc.tile_pool(name="snat", bufs=1) as snat_pool:
            s_nat = snat_pool.tile([NS, D], F32)
            for sc_ in range(8):
                nc.sync.dma_start(
                    s_nat[:, sc_ * 1024:(sc_ + 1) * 1024],
                    s_dram[:, sc_ * 1024:(sc_ + 1) * 1024])
            for c in range(DC):
                tp = tr_ps.tile([128, NS], F32)
                nc.tensor.transpose(tp[:], s_nat[:, c * 128:(c + 1) * 128],
                                    ident[:NS, :NS])
                if c % 2 == 0:
                    nc.scalar.copy(stq[:, c * NS:(c + 1) * NS], tp[:])
                else:
                    nc.vector.tensor_copy(stq[:, c * NS:(c + 1) * NS], tp[:])
            kp = cpt[:, NQC:NQC + NS]
            for c in range(DC):
                nc.tensor.matmul(kp, stq[:, c * NS:(c + 1) * NS],
                                 stq[:, c * NS:(c + 1) * NS],
                                 start=(c == 0), stop=(c == DC - 1))
            K_sb = persist.tile([NS, NS], F32)
            nc.scalar.copy(K_sb[:], kp)

        # Kzd = K with zeroed diagonal; diagKp1 = diag(K) + 1
        kdiag = persist.tile([NS, NS], F32)
        dK = persist.tile([NS, 1], F32)
        nc.vector.scalar_tensor_tensor(kdiag[:], K_sb[:], 1.0, ident[:NS, :NS],
                                       op0=ALU.mult, op1=ALU.mult,
                                       accum_out=dK[:])
        Kzd = persist.tile([NS, NS], F32)
        nc.vector.tensor_tensor(Kzd[:], K_sb[:], kdiag[:], op=ALU.subtract)
        dKp1 = persist.tile([NS, 1], F32)
        nc.vector.tensor_scalar_add(dKp1[:], dK[:], 1.0)

        # ---------------- phase C emit helper (independent of QP) -------
        # Both query row-blocks transposed into one [128d, 256q] rhs per
        # d-chunk -> a single compat matmul per chunk (Sᵀ weights loaded once)
        def emit_stream():
            for bc in range(4):
                qins = []
                for qt in range(2):
                    qin = qio.tile([128, 2048], F32, tag=f"qin{qt}")
                    nc.sync.dma_start(
                        qin[:], q_dram[qt * 128:(qt + 1) * 128,
                                       bc * 2048:(bc + 1) * 2048])
                    qins.append(qin)
                for sub in range(16):
                    c = bc * 16 + sub
                    qt_sb = qtp.tile([128, 256], F32)
                    for qt in range(2):
                        tp = tr_ps.tile([128, 128], F32)
                        nc.tensor.transpose(
                            tp[:], qins[qt][:, sub * 128:(sub + 1) * 128],
                            ident[:])
                        if (2 * c + qt) % 2 == 0:
                            nc.scalar.copy(qt_sb[:, qt * 128:(qt + 1) * 128],
                                           tp[:])
                        else:
                            nc.vector.tensor_copy(
                                qt_sb[:, qt * 128:(qt + 1) * 128], tp[:])
                    nc.tensor.matmul(cpt[:, 0:NQC],
                                     stq[:, c * NS:(c + 1) * NS], qt_sb[:],
                                     start=(c == 0), stop=(c == DC - 1))

        emit_stream()

        # ---------------- phase B: interior-point QP ----------------
        # state ST: cols 0:5 Z | 5:10 S | 10:15 L | 15:16 nu
        ST = persist.tile([NS, 16], F32)
        nc.gpsimd.memset(ST[:, 0:5], 0.0)
        nc.gpsimd.memset(ST[:, 5:15], 1.0)
        nc.gpsimd.memset(ST[:, 15:16], 0.0)
        DP = persist.tile([NS, 16], F32)
        X = persist.tile([NS, NW * NS], F32)         # five H_w^{-1}
        Hs = persist.tile([NS, NW * NS], F32)
        XS = persist.tile([NS, NS], F32)
        Ssch = persist.tile([NS, NS], F32)

        for it in range(IP_ITERS):
            Zc, Sc, Lc, nuc = ST[:, 0:5], ST[:, 5:10], ST[:, 10:15], ST[:, 15:16]
            # GZ = K @ Z   (K symmetric)
            gzp = mm_ps.tile([NS, NW], F32, tag="mm_small")
            nc.tensor.matmul(gzp[:], K_sb[:], Zc, start=True, stop=True)
            # SL = S*L with row sums
            sl = qp_tmp.tile([NS, NW], F32, tag="sl")
            slsum = qp_tmp.tile([NS, 1], F32, tag="slsum")
            nc.vector.scalar_tensor_tensor(sl[:], Sc, 1.0, Lc,
                                           op0=ALU.mult, op1=ALU.mult,
                                           accum_out=slsum[:])
            # mu = SIGMA/(ns*nw) * total(SL), broadcast to all partitions
            mup = mm_ps.tile([NS, 1], F32, tag="mm_small")
            nc.tensor.matmul(mup[:], ones_sq[:], slsum[:], start=True, stop=True)
            mu = qp_tmp.tile([NS, 1], F32, tag="mu")
            nc.vector.tensor_scalar_mul(mu[:], mup[:], SIGMA / (NS * NW))
            # r3 = SL - mu
            r3 = qp_tmp.tile([NS, NW], F32, tag="r3")
            nc.vector.tensor_scalar(r3[:], sl[:], mu[:], None, op0=ALU.subtract)
            # sinv, dv = L/s, dkd = diag(K)+1+dv, g2 = 1/dkd
            sinv = qp_tmp.tile([NS, NW], F32, tag="sinv")
            nc.vector.reciprocal(sinv[:], Sc)
            fresh, n_ns = NS_SCHED[it]
            dv = qp_tmp.tile([NS, NW], F32, tag="dv")
            nc.vector.tensor_tensor(dv[:], Lc, sinv[:], op=ALU.mult)
            if n_ns > 0:
                dkd = qp_tmp.tile([NS, NW], F32, tag="dkd")
                nc.vector.tensor_scalar(dkd[:], dv[:], dKp1[:], None,
                                        op0=ALU.add)
            if fresh:
                g2 = qp_tmp.tile([NS, NW], F32, tag="g2")
                nc.vector.reciprocal(g2[:], dkd[:])
            # r1 = GZ + Z - y1h + L + nu
            r1 = qp_tmp.tile([NS, NW], F32, tag="r1")
            nc.vector.scalar_tensor_tensor(r1[:], gzp[:], nuc, Zc,
                                           op0=ALU.add, op1=ALU.add)
            nc.vector.tensor_tensor(r1[:], r1[:], y1[:], op=ALU.subtract)
            nc.vector.tensor_tensor(r1[:], r1[:], Lc, op=ALU.add)
            # r2 = Z + S - h
            r2 = qp_tmp.tile([NS, NW], F32, tag="r2")
            nc.vector.scalar_tensor_tensor(r2[:], Zc, 1.0, Sc,
                                           op0=ALU.mult, op1=ALU.add)
            nc.vector.tensor_tensor(r2[:], r2[:], hC[:], op=ALU.subtract)
            # r4 = rowsum(Z)
            r4 = qp_tmp.tile([NS, 1], F32, tag="r4")
            nc.vector.tensor_reduce(r4[:], Zc, axis=AXL.X, op=ALU.add)
            # rhs1 = -(r1 + (L*r2 - r3)/s)
            t5 = qp_tmp.tile([NS, NW], F32, tag="t5")
            nc.vector.tensor_tensor(t5[:], Lc, r2[:], op=ALU.mult)
            nc.vector.tensor_tensor(t5[:], t5[:], r3[:], op=ALU.subtract)
            nc.vector.tensor_tensor(t5[:], t5[:], sinv[:], op=ALU.mult)
            rhs1 = qp_tmp.tile([NS, NW], F32, tag="rhs1")
            nc.vector.scalar_tensor_tensor(rhs1[:], t5[:], -1.0, r1[:],
                                           op0=ALU.mult, op1=ALU.subtract)
            # H_w = Kzd + diag(dkd_w); X0_w = diag(g2_w) on fresh iters
            for w in range(NW):
                ws = slice(w * NS, (w + 1) * NS)
                if n_ns > 0:
                    nc.vector.scalar_tensor_tensor(
                        Hs[:, ws], ident[:NS, :NS], dkd[:, w:w + 1], Kzd[:],
                        op0=ALU.mult, op1=ALU.add)
                if fresh:
                    nc.vector.tensor_scalar(X[:, ws], ident[:NS, :NS],
                                            g2[:, w:w + 1], None, op0=ALU.mult)
            # Newton-Schulz: X <- 2X - X (H X)
            for _ in range(n_ns):
                for w in range(NW):
                    ws = slice(w * NS, (w + 1) * NS)
                    yp = y_ps.tile([NS, NS], F32)
                    nc.tensor.matmul(yp[:], Hs[:, ws], X[:, ws],
                                     start=True, stop=True)
                    ysb = ns_sb.tile([NS, NS], F32, tag="ysb")
                    nc.vector.tensor_copy(ysb[:], yp[:])
                    xyp = xy_ps.tile([NS, NS], F32)
                    nc.tensor.matmul(xyp[:], X[:, ws], ysb[:],
                                     start=True, stop=True)
                    nc.vector.scalar_tensor_tensor(X[:, ws], X[:, ws], 2.0,
                                                   xyp[:], op0=ALU.mult,
                                                   op1=ALU.subtract)
            # usum = sum_w X_w rhs1_w (PSUM-accumulated across the 5 mms)
            up = mm_ps.tile([NS, 1], F32, tag="mm_small")
            for w in range(NW):
                nc.tensor.matmul(up[:], X[:, w * NS:(w + 1) * NS],
                                 rhs1[:, w:w + 1], start=(w == 0),
                                 stop=(w == NW - 1))
            # Schur = sum_w X_w ; Jacobi init for its inverse
            if n_ns > 0:
                xv = X[:].rearrange("p (w j) -> p j w", w=NW)
                nc.vector.tensor_reduce(Ssch[:], xv, axis=AXL.X, op=ALU.add)
            if fresh:
                dSg = qp_tmp.tile([NS, 1], F32, tag="dSg")
                schd = qp_tmp.tile([NS, NS], F32, tag="schd")
                nc.vector.scalar_tensor_tensor(schd[:], Ssch[:], 1.0,
                                               ident[:NS, :NS], op0=ALU.mult,
                                               op1=ALU.mult, accum_out=dSg[:])
                gs2 = qp_tmp.tile([NS, 1], F32, tag="gs2")
                nc.vector.reciprocal(gs2[:], dSg[:])
                nc.vector.tensor_scalar(XS[:], ident[:NS, :NS], gs2[:], None,
                                        op0=ALU.mult)
            for _ in range(n_ns):
                yp = y_ps.tile([NS, NS], F32)
                nc.tensor.matmul(yp[:], Ssch[:], XS[:], start=True, stop=True)
                ysb = ns_sb.tile([NS, NS], F32, tag="ysb")
                nc.scalar.copy(ysb[:], yp[:])
                xyp = xy_ps.tile([NS, NS], F32)
                nc.tensor.matmul(xyp[:], XS[:], ysb[:], start=True, stop=True)
                nc.vector.scalar_tensor_tensor(XS[:], XS[:], 2.0, xyp[:],
                                               op0=ALU.mult, op1=ALU.subtract)
            # rhs_s = usum + r4 ; dnu = XS rhs_s
            rhss = qp_tmp.tile([NS, 1], F32, tag="rhss")
            nc.vector.tensor_tensor(rhss[:], up[:], r4[:], op=ALU.add)
            dnup = mm_ps.tile([NS, 1], F32, tag="mm_small")
            nc.tensor.matmul(dnup[:], XS[:], rhss[:], start=True, stop=True)
            nc.vector.tensor_copy(DP[:, 15:16], dnup[:])
            # dZ = X (rhs1 - dnu broadcast over w)
            rhs2 = qp_tmp.tile([NS, NW], F32, tag="rhs2")
            nc.vector.tensor_scalar(rhs2[:], rhs1[:], DP[:, 15:16], None,
                                    op0=ALU.subtract)
            xdp = mm_ps.tile([NS, NW], F32, tag="mm_small")
            for w in range(NW):
                nc.tensor.matmul(xdp[:, w:w + 1], X[:, w * NS:(w + 1) * NS],
                                 rhs2[:, w:w + 1], start=True, stop=True)
            nc.vector.tensor_copy(DP[:, 0:5], xdp[:])
            # dS = -dZ - r2 ; dL = dZ*(L/s) + (L*r2 - r3)/s  (reuses t5)
            nc.vector.scalar_tensor_tensor(DP[:, 5:10], DP[:, 0:5], -1.0,
                                           r2[:], op0=ALU.mult,
                                           op1=ALU.subtract)
            dl = qp_tmp.tile([NS, NW], F32, tag="dl")
            nc.vector.tensor_tensor(dl[:], DP[:, 0:5], dv[:], op=ALU.mult)
            nc.vector.tensor_tensor(DP[:, 10:15], dl[:], t5[:], op=ALU.add)
            # alpha = min(1, 0.99 / max(-d(S|L)/(S|L), eps))
            xpi = qp_tmp.tile([NS, 10], F32, tag="xpi")
            nc.vector.reciprocal(xpi[:], ST[:, 5:15])
            qr_ = qp_tmp.tile([NS, 10], F32, tag="qr_")
            nc.vector.scalar_tensor_tensor(qr_[:], DP[:, 5:15], -1.0, xpi[:],
                                           op0=ALU.mult, op1=ALU.mult)
            # global max: free-reduce, one transpose, free-reduce
            qm = qp_tmp.tile([NS, 1], F32, tag="qm")
            nc.vector.tensor_reduce(qm[:], qr_[:], axis=AXL.X, op=ALU.max)
            qtr = mm_ps.tile([1, NS], F32, tag="mm_small")
            nc.tensor.transpose(qtr[:], qm[:], ident[:NS, :NS])
            qtr_sb = qp_tmp.tile([1, NS], F32, tag="qtr_sb")
            nc.vector.tensor_copy(qtr_sb[:], qtr[:])
            am = qp_tmp.tile([1, 1], F32, tag="am")
            nc.vector.tensor_reduce(am[:], qtr_sb[:], axis=AXL.X, op=ALU.max)
            nc.vector.tensor_scalar_max(am[:], am[:], 1e-30)
            nc.vector.reciprocal(am[:], am[:])
            nc.vector.tensor_scalar(am[:], am[:], 0.99, 1.0,
                                    op0=ALU.mult, op1=ALU.min)
            alp = mm_ps.tile([NS, 1], F32, tag="mm_small")
            nc.tensor.matmul(alp[:], ones_row[:1, :NS], am[:],
                             start=True, stop=True)
            asb = qp_tmp.tile([NS, 1], F32, tag="asb")
            nc.vector.tensor_copy(asb[:], alp[:])
            # state += alpha * delta
            nc.vector.scalar_tensor_tensor(ST[:], DP[:], asb[:], ST[:],
                                           op0=ALU.mult, op1=ALU.add)

        # ---------------- num_sv ----------------
        cmp_ = qp_tmp.tile([NS, NW], F32, tag="cmp_")
        nc.vector.tensor_scalar(cmp_[:], ST[:, 0:5], SV_THRESH, None,
                                op0=ALU.is_gt)
        anyw = qp_tmp.tile([NS, 1], F32, tag="anyw")
        nc.vector.tensor_reduce(anyw[:], cmp_[:], axis=AXL.X, op=ALU.max)
        nsp = mm_ps.tile([NS, 1], F32, tag="mm_small")
        nc.tensor.matmul(nsp[:], ones_sq[:], anyw[:], start=True, stop=True)
        aux_sb = persist.tile([1, 4], F32)
        nc.gpsimd.memset(aux_sb[:], 0.0)
        nc.scalar.copy(aux_sb[0:1, 0:1], nsp[0:1, :])
        nc.sync.dma_start(aux_dram[:], aux_sb[:])

        # ---------------- logits = compat^T @ Z, scaled ----------------
        cp_sb = persist.tile([NS, NQC], F32)
        nc.scalar.copy(cp_sb[:], cpt[:, 0:NQC])
        for qt in range(2):
            lgp = mm_ps.tile([128, NW], F32, tag="mm_small")
            nc.tensor.matmul(lgp[:], cp_sb[:, qt * 128:(qt + 1) * 128],
                             ST[:, 0:5], start=True, stop=True)
            lg_sb = qp_tmp.tile([128, NW], F32, tag="lg_sb")
            nc.vector.tensor_scalar_mul(lg_sb[:], lgp[:], sc_bc[:])
            nc.sync.dma_start(lg_dram[qt * 128:(qt + 1) * 128, :], lg_sb[:])


_NC_CACHE = None


def _get_nc():
    global _NC_CACHE
    if _NC_CACHE is None:
        _NC_CACHE = build_nc()
    return _NC_CACHE


def _prep_in_maps(query, support, support_labels, scale):
    query = np.ascontiguousarray(np.asarray(query, np.float32))
    support = np.ascontiguousarray(np.asarray(support, np.float32))
    labels = np.asarray(support_labels).astype(np.int64)
    y1h = np.zeros((NS, NW), np.float32)
    y1h[np.arange(NS), labels] = 1.0
    sc = np.asarray(scale, np.float32).reshape(1, 1)
    return [
        {
            "qslice": query[c * NQC:(c + 1) * NQC],
            "support": support,
            "y1h": y1h,
            "scale": sc,
        }
        for c in range(N_CORES)
    ]


def run_device(query, support, support_labels, scale, trace=False, **trace_kw):
    nc = _get_nc()
    in_maps = _prep_in_maps(query, support, support_labels, scale)
    res = run_bass_kernel_spmd(nc, in_maps, list(range(N_CORES)),
                               trace=trace, **trace_kw)
    logits = np.concatenate(
        [res.results[c]["logits"] for c in range(N_CORES)], axis=0)
    num_sv = res.results[0]["aux"][0, 0]
    return logits, num_sv, res


def kernel(query, support, support_labels, n_way, n_shot, scale):
    assert int(n_way) == NW and int(n_shot) * NW == NS
    logits, num_sv, _ = run_device(query, support, support_labels, scale)
    logits_full = logits.reshape(1, NQ, NW).astype(np.float32)
    return logits_full, np.int32(round(float(num_sv)))


# revision 36
# speedup vs baseline: 1.4561x; 1.0415x over previous
"""MetaOptNet SVM-CS classification head on 8 Trainium2 NeuronCores.

Sharding: data-parallel over query rows (2048 -> 8 x 256). Each core:
  - computes the support Gram matrix K = S S^T (contraction over d=8192
    via PE-transposed support chunks), replicated;
  - runs the full interior-point QP (14 Newton steps) replicated,
    exploiting that the KKT matrix H = kron(K,I5)+I+diag(lam/s) is
    block-diagonal over the 5 classes: five 125x125 SPD solves plus a
    125x125 Schur complement for the equality constraints. All solves
    use Newton-Schulz inversion with fresh Jacobi initialization
    (rho(I - X0 H) <= ~0.3 uniformly along the trajectory);
  - streams its query slice, PE-transposes chunks, accumulates
    compat = S Q_c^T in PSUM (overlaps the QP);
  - logits_c = compat^T @ qp3, scaled by `scale`.
Host only shards inputs, concatenates the 8 logits slices, and casts
num_sv (computed on device) to int32.
"""
import sys

for _p in ("/opt/trn_rl_repo", "/root/.axon_site/_ro/trn_rl_repo"):
    if _p not in sys.path:
        sys.path.append(_p)

import numpy as np

import concourse.bacc as bacc
import concourse.mybir as mybir
import concourse.tile as tile
from concourse.bass_utils import run_bass_kernel_spmd
from concourse.masks import make_identity

F32 = mybir.dt.float32
ALU = mybir.AluOpType
AXL = mybir.AxisListType

N_CORES = 8
NQ, D, NS, NW = 2048, 8192, 125, 5
NQC = NQ // N_CORES          # 256 query rows per core
DC = D // 128                # 64 d-chunks
C_REG, SIGMA = 0.1, 0.1
# Per-iteration (reinit, newton_schulz_steps) for the H/Schur inverses:
# fresh Jacobi + 2 NS through the lam/s spike, warm-started 1 NS while it
# decays, frozen inverses for the converged tail.
NS_SCHED = [(True, 2)] * 4 + [(False, 1)] * 5 + [(False, 0)] * 3
IP_ITERS = len(NS_SCHED)
SV_THRESH = 0.001


def build_nc():
    nc = bacc.Bacc("TRN2", target_bir_lowering=False, debug=False,
                   num_devices=N_CORES)
    q_dram = nc.dram_tensor("qslice", [NQC, D], F32, kind="ExternalInput")
    s_dram = nc.dram_tensor("support", [NS, D], F32, kind="ExternalInput")
    y_dram = nc.dram_tensor("y1h", [NS, NW], F32, kind="ExternalInput")
    sc_dram = nc.dram_tensor("scale", [1, 1], F32, kind="ExternalInput")
    lg_dram = nc.dram_tensor("logits", [NQC, NW], F32, kind="ExternalOutput")
    aux_dram = nc.dram_tensor("aux", [1, 4], F32, kind="ExternalOutput")

    with tile.TileContext(nc) as tc:
        _build(tc, q_dram, s_dram, y_dram, sc_dram, lg_dram, aux_dram)
    nc.compile()
    return nc


def _build(tc, q_dram, s_dram, y_dram, sc_dram, lg_dram, aux_dram):
    nc = tc.nc
    from contextlib import ExitStack

    ctx = ExitStack()
    with ctx:
        persist = ctx.enter_context(tc.tile_pool(name="persist", bufs=1))
        qp_tmp = ctx.enter_context(tc.tile_pool(name="qp_tmp", bufs=2))
        ns_sb = ctx.enter_context(tc.tile_pool(name="ns_sb", bufs=2))
        qio = ctx.enter_context(tc.tile_pool(name="qio", bufs=3))
        qtp = ctx.enter_context(tc.tile_pool(name="qtp", bufs=3))
        tr_ps = ctx.enter_context(tc.tile_pool(name="tr_ps", bufs=2, space="PSUM"))
        mm_ps = ctx.enter_context(tc.tile_pool(name="mm_ps", bufs=1, space="PSUM"))
        y_ps = ctx.enter_context(tc.tile_pool(name="y_ps", bufs=2, space="PSUM"))
        xy_ps = ctx.enter_context(tc.tile_pool(name="xy_ps", bufs=2, space="PSUM"))
        cp_ps = ctx.enter_context(tc.tile_pool(name="cp_ps", bufs=1, space="PSUM"))

        # ---------------- constants ----------------
        ident = persist.tile([128, 128], F32)
        make_identity(nc, ident)
        ones_sq = persist.tile([NS, NS], F32)
        nc.gpsimd.memset(ones_sq[:], 1.0)
        ones_row = persist.tile([1, 128], F32)
        nc.gpsimd.memset(ones_row[:], 1.0)

        y1 = persist.tile([NS, NW], F32)
        nc.sync.dma_start(y1[:], y_dram[:])
        hC = persist.tile([NS, NW], F32)
        nc.vector.tensor_scalar_mul(hC[:], y1[:], C_REG)

        sc_sb = persist.tile([1, 1], F32)
        nc.sync.dma_start(sc_sb[:], sc_dram[:])
        scp = mm_ps.tile([128, 1], F32, tag="mm_small")
        nc.tensor.matmul(scp[:], ones_row[:], sc_sb[:], start=True, stop=True)
        sc_bc = persist.tile([128, 1], F32)
        nc.scalar.copy(sc_bc[:], scp[:])

        # compat + K share one PSUM bank: cols 0:256 compat, 256:381 K
        cpt = cp_ps.tile([NS, NQC + NS], F32)

        # ---------------- phase A: support load, S^T, K ----------------
        stq = persist.tile([128, DC * NS], F32)      # S^T chunks [128d, 125]
        with tc.tile_pool(name="snat", bufs=1) as snat_pool:
            s_nat = snat_pool.tile([NS, D], F32)
            for sc_ in range(8):
                nc.sync.dma_start(
                    s_nat[:, sc_ * 1024:(sc_ + 1) * 1024],
                    s_dram[:, sc_ * 1024:(sc_ + 1) * 1024])
            for c in range(DC):
                tp = tr_ps.tile([128, NS], F32)
                nc.tensor.transpose(tp[:], s_nat[:, c * 128:(c + 1) * 128],
                                    ident[:NS, :NS])
                if c % 2 == 0:
                    nc.scalar.copy(stq[:, c * NS:(c + 1) * NS], tp[:])
                else:
                    nc.vector.tensor_copy(stq[:, c * NS:(c + 1) * NS], tp[:])
            kp = cpt[:, NQC:NQC + NS]
            for c in range(DC):
                nc.tensor.matmul(kp, stq[:, c * NS:(c + 1) * NS],
                                 stq[:, c * NS:(c + 1) * NS],
                                 start=(c == 0), stop=(c == DC - 1))
            K_sb = persist.tile([NS, NS], F32)
            nc.scalar.copy(K_sb[:], kp)

        # Kzd = K with zeroed diagonal; diagKp1 = diag(K) + 1
        kdiag = persist.tile([NS, NS], F32)
        dK = persist.tile([NS, 1], F32)
        nc.vector.scalar_tensor_tensor(kdiag[:], K_sb[:], 1.0, ident[:NS, :NS],
                                       op0=ALU.mult, op1=ALU.mult,
                                       accum_out=dK[:])
        Kzd = persist.tile([NS, NS], F32)
        nc.vector.tensor_tensor(Kzd[:], K_sb[:], kdiag[:], op=ALU.subtract)
        dKp1 = persist.tile([NS, 1], F32)
        nc.vector.tensor_scalar_add(dKp1[:], dK[:], 1.0)

        # ---------------- phase C emit helper (independent of QP) -------
        # Both query row-blocks transposed into one [128d, 256q] rhs per
        # d-chunk -> a single compat matmul per chunk (Sᵀ weights loaded once)
        def emit_stream():
            for bc in range(4):
                qins = []
                for qt in range(2):
                    qin = qio.tile([128, 2048], F32, tag=f"qin{qt}")
                    nc.sync.dma_start(
                        qin[:], q_dram[qt * 128:(qt + 1) * 128,
                                       bc * 2048:(bc + 1) * 2048])
                    qins.append(qin)
                for sub in range(16):
                    c = bc * 16 + sub
                    qt_sb = qtp.tile([128, 256], F32)
                    for qt in range(2):
                        tp = tr_ps.tile([128, 128], F32)
                        nc.tensor.transpose(
                            tp[:], qins[qt][:, sub * 128:(sub + 1) * 128],
                            ident[:])
                        if (2 * c + qt) % 2 == 0:
                            nc.scalar.copy(qt_sb[:, qt * 128:(qt + 1) * 128],
                                           tp[:])
                        else:
                            nc.vector.tensor_copy(
                                qt_sb[:, qt * 128:(qt + 1) * 128], tp[:])
                    nc.tensor.matmul(cpt[:, 0:NQC],
                                     stq[:, c * NS:(c + 1) * NS], qt_sb[:],
                                     start=(c == 0), stop=(c == DC - 1))

        emit_stream()

        # ---------------- phase B: interior-point QP ----------------
        # state ST: cols 0:5 Z | 5:10 S | 10:15 L | 15:16 nu
        ST = persist.tile([NS, 16], F32)
        nc.gpsimd.memset(ST[:, 0:5], 0.0)
        nc.gpsimd.memset(ST[:, 5:15], 1.0)
        nc.gpsimd.memset(ST[:, 15:16], 0.0)
        DP = persist.tile([NS, 16], F32)
        X = persist.tile([NS, NW * NS], F32)         # five H_w^{-1}
        XS = persist.tile([NS, NS], F32)
        Ssch = persist.tile([NS, NS], F32)

        for it in range(IP_ITERS):
            Zc, Sc, Lc, nuc = ST[:, 0:5], ST[:, 5:10], ST[:, 10:15], ST[:, 15:16]
            # GZ = K @ Z   (K symmetric)
            gzp = mm_ps.tile([NS, NW], F32, tag="mm_small")
            nc.tensor.matmul(gzp[:], K_sb[:], Zc, start=True, stop=True)
            # SL = S*L with row sums
            sl = qp_tmp.tile([NS, NW], F32, tag="sl")
            slsum = qp_tmp.tile([NS, 1], F32, tag="slsum")
            nc.vector.scalar_tensor_tensor(sl[:], Sc, 1.0, Lc,
                                           op0=ALU.mult, op1=ALU.mult,
                                           accum_out=slsum[:])
            # mu = SIGMA/(ns*nw) * total(SL), broadcast to all partitions
            mup = mm_ps.tile([NS, 1], F32, tag="mm_small")
            nc.tensor.matmul(mup[:], ones_sq[:], slsum[:], start=True, stop=True)
            mu = qp_tmp.tile([NS, 1], F32, tag="mu")
            nc.vector.tensor_scalar_mul(mu[:], mup[:], SIGMA / (NS * NW))
            # r3 = SL - mu
            r3 = qp_tmp.tile([NS, NW], F32, tag="r3")
            nc.vector.tensor_scalar(r3[:], sl[:], mu[:], None, op0=ALU.subtract)
            # sinv, dv = L/s, dkd = diag(K)+1+dv, g2 = 1/dkd
            sinv = qp_tmp.tile([NS, NW], F32, tag="sinv")
            nc.vector.reciprocal(sinv[:], Sc)
            fresh, n_ns = NS_SCHED[it]
            dv = qp_tmp.tile([NS, NW], F32, tag="dv")
            nc.vector.tensor_tensor(dv[:], Lc, sinv[:], op=ALU.mult)
            if n_ns > 0:
                dkd = qp_tmp.tile([NS, NW], F32, tag="dkd")
                nc.vector.tensor_scalar(dkd[:], dv[:], dKp1[:], None,
                                        op0=ALU.add)
            if fresh:
                g2 = qp_tmp.tile([NS, NW], F32, tag="g2")
                nc.vector.reciprocal(g2[:], dkd[:])
            # r1 = GZ + Z - y1h + L + nu
            r1 = qp_tmp.tile([NS, NW], F32, tag="r1")
            nc.vector.scalar_tensor_tensor(r1[:], gzp[:], nuc, Zc,
                                           op0=ALU.add, op1=ALU.add)
            nc.vector.tensor_tensor(r1[:], r1[:], y1[:], op=ALU.subtract)
            nc.vector.tensor_tensor(r1[:], r1[:], Lc, op=ALU.add)
            # r2 = Z + S - h
            r2 = qp_tmp.tile([NS, NW], F32, tag="r2")
            nc.vector.scalar_tensor_tensor(r2[:], Zc, 1.0, Sc,
                                           op0=ALU.mult, op1=ALU.add)
            nc.vector.tensor_tensor(r2[:], r2[:], hC[:], op=ALU.subtract)
            # r4 = rowsum(Z)
            r4 = qp_tmp.tile([NS, 1], F32, tag="r4")
            nc.vector.tensor_reduce(r4[:], Zc, axis=AXL.X, op=ALU.add)
            # rhs1 = -(r1 + (L*r2 - r3)/s)
            t5 = qp_tmp.tile([NS, NW], F32, tag="t5")
            nc.vector.tensor_tensor(t5[:], Lc, r2[:], op=ALU.mult)
            nc.vector.tensor_tensor(t5[:], t5[:], r3[:], op=ALU.subtract)
            nc.vector.tensor_tensor(t5[:], t5[:], sinv[:], op=ALU.mult)
            rhs1 = qp_tmp.tile([NS, NW], F32, tag="rhs1")
            nc.vector.scalar_tensor_tensor(rhs1[:], t5[:], -1.0, r1[:],
                                           op0=ALU.mult, op1=ALU.subtract)
            # X0_w = diag(g2_w) on fresh iters
            if fresh:
                for w in range(NW):
                    ws = slice(w * NS, (w + 1) * NS)
                    nc.vector.tensor_scalar(X[:, ws], ident[:NS, :NS],
                                            g2[:, w:w + 1], None, op0=ALU.mult)
            # Newton-Schulz: X <- 2X - X (H X) with H_w X_w formed as
            # Kzd @ X_w (shared stationary weights) + dkd_w-row-scaled X_w
            # fused into the PSUM eviction -- H is never materialized.
            for _ in range(n_ns):
                for w in range(NW):
                    ws = slice(w * NS, (w + 1) * NS)
                    yp = y_ps.tile([NS, NS], F32)
                    nc.tensor.matmul(yp[:], Kzd[:], X[:, ws],
                                     start=True, stop=True)
                    ysb = ns_sb.tile([NS, NS], F32, tag="ysb")
                    nc.vector.scalar_tensor_tensor(ysb[:], X[:, ws],
                                                   dkd[:, w:w + 1], yp[:],
                                                   op0=ALU.mult, op1=ALU.add)
                    xyp = xy_ps.tile([NS, NS], F32)
                    nc.tensor.matmul(xyp[:], X[:, ws], ysb[:],
                                     start=True, stop=True)
                    nc.vector.scalar_tensor_tensor(X[:, ws], X[:, ws], 2.0,
                                                   xyp[:], op0=ALU.mult,
                                                   op1=ALU.subtract)
            # usum = sum_w X_w rhs1_w (PSUM-accumulated across the 5 mms)
            up = mm_ps.tile([NS, 1], F32, tag="mm_small")
            for w in range(NW):
                nc.tensor.matmul(up[:], X[:, w * NS:(w + 1) * NS],
                                 rhs1[:, w:w + 1], start=(w == 0),
                                 stop=(w == NW - 1))
            # Schur = sum_w X_w ; Jacobi init for its inverse
            if n_ns > 0:
                xv = X[:].rearrange("p (w j) -> p j w", w=NW)
                nc.vector.tensor_reduce(Ssch[:], xv, axis=AXL.X, op=ALU.add)
            if fresh:
                dSg = qp_tmp.tile([NS, 1], F32, tag="dSg")
                schd = qp_tmp.tile([NS, NS], F32, tag="schd")
                nc.vector.scalar_tensor_tensor(schd[:], Ssch[:], 1.0,
                                               ident[:NS, :NS], op0=ALU.mult,
                                               op1=ALU.mult, accum_out=dSg[:])
                gs2 = qp_tmp.tile([NS, 1], F32, tag="gs2")
                nc.vector.reciprocal(gs2[:], dSg[:])
                nc.vector.tensor_scalar(XS[:], ident[:NS, :NS], gs2[:], None,
                                        op0=ALU.mult)
            for _ in range(n_ns):
                yp = y_ps.tile([NS, NS], F32)
                nc.tensor.matmul(yp[:], Ssch[:], XS[:], start=True, stop=True)
                ysb = ns_sb.tile([NS, NS], F32, tag="ysb")
                nc.scalar.copy(ysb[:], yp[:])
                xyp = xy_ps.tile([NS, NS], F32)
                nc.tensor.matmul(xyp[:], XS[:], ysb[:], start=True, stop=True)
                nc.vector.scalar_tensor_tensor(XS[:], XS[:], 2.0, xyp[:],
                                               op0=ALU.mult, op1=ALU.subtract)
            # rhs_s = usum + r4 ; dnu = XS rhs_s
            rhss = qp_tmp.tile([NS, 1], F32, tag="rhss")
            nc.vector.tensor_tensor(rhss[:], up[:], r4[:], op=ALU.add)
            dnup = mm_ps.tile([NS, 1], F32, tag="mm_small")
            nc.tensor.matmul(dnup[:], XS[:], rhss[:], start=True, stop=True)
            nc.vector.tensor_copy(DP[:, 15:16], dnup[:])
            # dZ = X (rhs1 - dnu broadcast over w)
            rhs2 = qp_tmp.tile([NS, NW], F32, tag="rhs2")
            nc.vector.tensor_scalar(rhs2[:], rhs1[:], DP[:, 15:16], None,
                                    op0=ALU.subtract)
            xdp = mm_ps.tile([NS, NW], F32, tag="mm_small")
            for w in range(NW):
                nc.tensor.matmul(xdp[:, w:w + 1], X[:, w * NS:(w + 1) * NS],
                                 rhs2[:, w:w + 1], start=True, stop=True)
            nc.vector.tensor_copy(DP[:, 0:5], xdp[:])
            # dS = -dZ - r2 ; dL = dZ*(L/s) + (L*r2 - r3)/s  (reuses t5)
            nc.vector.scalar_tensor_tensor(DP[:, 5:10], DP[:, 0:5], -1.0,
                                           r2[:], op0=ALU.mult,
                                           op1=ALU.subtract)
            dl = qp_tmp.tile([NS, NW], F32, tag="dl")
            nc.vector.tensor_tensor(dl[:], DP[:, 0:5], dv[:], op=ALU.mult)
            nc.vector.tensor_tensor(DP[:, 10:15], dl[:], t5[:], op=ALU.add)
            # alpha = min(1, 0.99 / max(-d(S|L)/(S|L), eps))
            xpi = qp_tmp.tile([NS, 10], F32, tag="xpi")
            nc.vector.reciprocal(xpi[:], ST[:, 5:15])
            qr_ = qp_tmp.tile([NS, 10], F32, tag="qr_")
            nc.vector.scalar_tensor_tensor(qr_[:], DP[:, 5:15], -1.0, xpi[:],
                                           op0=ALU.mult, op1=ALU.mult)
            # global max: free-reduce, one transpose, free-reduce
            qm = qp_tmp.tile([NS, 1], F32, tag="qm")
            nc.vector.tensor_reduce(qm[:], qr_[:], axis=AXL.X, op=ALU.max)
            qtr = mm_ps.tile([1, NS], F32, tag="mm_small")
            nc.tensor.transpose(qtr[:], qm[:], ident[:NS, :NS])
            qtr_sb = qp_tmp.tile([1, NS], F32, tag="qtr_sb")
            nc.vector.tensor_copy(qtr_sb[:], qtr[:])
            am = qp_tmp.tile([1, 1], F32, tag="am")
            nc.vector.tensor_reduce(am[:], qtr_sb[:], axis=AXL.X, op=ALU.max)
            nc.vector.tensor_scalar_max(am[:], am[:], 1e-30)
            nc.vector.reciprocal(am[:], am[:])
            nc.vector.tensor_scalar(am[:], am[:], 0.99, 1.0,
                                    op0=ALU.mult, op1=ALU.min)
            alp = mm_ps.tile([NS, 1], F32, tag="mm_small")
            nc.tensor.matmul(alp[:], ones_row[:1, :NS], am[:],
                             start=True, stop=True)
            asb = qp_tmp.tile([NS, 1], F32, tag="asb")
            nc.vector.tensor_copy(asb[:], alp[:])
            # state += alpha * delta
            nc.vector.scalar_tensor_tensor(ST[:], DP[:], asb[:], ST[:],
                                           op0=ALU.mult, op1=ALU.add)

        # ---------------- num_sv ----------------
        cmp_ = qp_tmp.tile([NS, NW], F32, tag="cmp_")
        nc.vector.tensor_scalar(cmp_[:], ST[:, 0:5], SV_THRESH, None,
                                op0=ALU.is_gt)
        anyw = qp_tmp.tile([NS, 1], F32, tag="anyw")
        nc.vector.tensor_reduce(anyw[:], cmp_[:], axis=AXL.X, op=ALU.max)
        nsp = mm_ps.tile([NS, 1], F32, tag="mm_small")
        nc.tensor.matmul(nsp[:], ones_sq[:], anyw[:], start=True, stop=True)
        aux_sb = persist.tile([1, 4], F32)
        nc.gpsimd.memset(aux_sb[:], 0.0)
        nc.scalar.copy(aux_sb[0:1, 0:1], nsp[0:1, :])
        nc.sync.dma_start(aux_dram[:], aux_sb[:])

        # ---------------- logits = compat^T @ Z, scaled ----------------
        cp_sb = persist.tile([NS, NQC], F32)
        nc.scalar.copy(cp_sb[:], cpt[:, 0:NQC])
        for qt in range(2):
            lgp = mm_ps.tile([128, NW], F32, tag="mm_small")
            nc.tensor.matmul(lgp[:], cp_sb[:, qt * 128:(qt + 1) * 128],
                             ST[:, 0:5], start=True, stop=True)
            lg_sb = qp_tmp.tile([128, NW], F32, tag="lg_sb")
            nc.vector.tensor_scalar_mul(lg_sb[:], lgp[:], sc_bc[:])
            nc.sync.dma_start(lg_dram[qt * 128:(qt + 1) * 128, :], lg_sb[:])


_NC_CACHE = None


def _get_nc():
    global _NC_CACHE
    if _NC_CACHE is None:
        _NC_CACHE = build_nc()
    return _NC_CACHE


def _prep_in_maps(query, support, support_labels, scale):
    query = np.ascontiguousarray(np.asarray(query, np.float32))
    support = np.ascontiguousarray(np.asarray(support, np.float32))
    labels = np.asarray(support_labels).astype(np.int64)
    y1h = np.zeros((NS, NW), np.float32)
    y1h[np.arange(NS), labels] = 1.0
    sc = np.asarray(scale, np.float32).reshape(1, 1)
    return [
        {
            "qslice": query[c * NQC:(c + 1) * NQC],
            "support": support,
            "y1h": y1h,
            "scale": sc,
        }
        for c in range(N_CORES)
    ]


def run_device(query, support, support_labels, scale, trace=False, **trace_kw):
    nc = _get_nc()
    in_maps = _prep_in_maps(query, support, support_labels, scale)
    res = run_bass_kernel_spmd(nc, in_maps, list(range(N_CORES)),
                               trace=trace, **trace_kw)
    logits = np.concatenate(
        [res.results[c]["logits"] for c in range(N_CORES)], axis=0)
    num_sv = res.results[0]["aux"][0, 0]
    return logits, num_sv, res


def kernel(query, support, support_labels, n_way, n_shot, scale):
    assert int(n_way) == NW and int(n_shot) * NW == NS
    logits, num_sv, _ = run_device(query, support, support_labels, scale)
    logits_full = logits.reshape(1, NQ, NW).astype(np.float32)
    return logits_full, np.int32(round(float(num_sv)))
